# revision 7
# baseline (speedup 1.0000x reference)
"""Trainium2 Bass kernel for a dense transformer block (pre-LN, 12 heads, MLP 4x).

Strategy: data-parallel over batch across the 8 NeuronCores (B=8 -> one batch
element per core, no collectives). Per core:

  - residual stream token-major f32 [128 tok x 8 chunks x 768] (DMA'd straight
    from HBM, no cast)
  - LN on DVE (bn_stats/bn_aggr); rstd = Exp(-0.5*Ln(var+eps)) on ACT so the
    whole kernel needs only the natural_log_exp table set plus one gelu load
  - fp8e4 (TRN e4m3, max 240) DoubleRow matmuls for qkv, ctx (E@V), proj and
    fc1: weights host-quantized at 32x scale, activations at unit scale;
    descales fold into the consuming DVE/ACT op. S (q@k) stays bf16; fc2 is
    fp8-DR or bf16 depending on the error budget (fc2_fp8 flag).
  - S (q@k, contraction = head_dim 64) uses PE row tiling: the two heads of a
    pair sit at partitions 0:64 / 64:128, so their matmuls land in row groups
    h0/h64 and run CONCURRENTLY in the PE array when interleaved - that is the
    point of the (i, ab) emission order in emit_s_exp.
  - softmax: S psum tiles [128, 2kc, 512q]; one exp per tile writing fp8 in
    the pair layout the DoubleRow ctx matmul consumes. Denominator via a
    1/32-scaled ones column in the stationary V operand; odd heads of a pair
    put v in cols 129..192 so their ctx lands on partitions 64..127 with the
    denominator on partition 0 - no cross-partition fixups. Normalization:
    reciprocal_approx_fast + gpsimd partition_broadcast + one DVE mult.
  - attention is processed in two q-halves so ctx/proj of half 0 overlap the
    exp stream of half 1. fc1+gelu+fc2 sit after the last exp because the
    gelu table set must not interleave with the exp set (2.7us reload per
    flap) and fc1 psum tiles serialize against their gelu.
  - bias matmuls (ones-row trick) are only emitted when the corresponding
    biases are nonzero; for this problem's inputs they are all zero.
"""

import numpy as np

import concourse.bass as bass
import concourse.mybir as mybir
import concourse.tile as tile
from concourse import bacc
from concourse.masks import make_identity

DIM = 768
HEADS = 12
HD = 64
HIDDEN = 3072
N_TOK = 1024
TC = N_TOK // 128  # 8 token chunks
FC = DIM // 128  # 6 feature chunks
KP = 3  # DoubleRow contraction pairs over 768
KP2 = HIDDEN // 256  # 12 DoubleRow pairs over 3072
MC_H = HIDDEN // 128  # 24 hidden chunks
EPS = 1e-5
SCALE = HD ** -0.5
WS = 32.0  # fp8 weight upscale
VP = 208  # per-head-pair free-dim stride in v_pair

F32 = mybir.dt.float32
BF16 = mybir.dt.bfloat16
F8 = mybir.dt.float8e4
DR = mybir.MatmulPerfMode.DoubleRow
DRI = mybir.MatmulPerfMode.DoubleRowSwInterleave


def build_bass(with_bias=True, fc2_fp8=False):
    nc = bacc.Bacc("TRN2", debug=False)

    x_d = nc.dram_tensor("x", [N_TOK, DIM], F32, kind="ExternalInput")
    wqk_d = nc.dram_tensor("wqk8i", [128, KP, 2 * FC, 256], F8, kind="ExternalInput")
    wv_d = nc.dram_tensor("wv8", [128, KP, 2, DIM], F8, kind="ExternalInput")
    wp_d = nc.dram_tensor("wp8", [128, KP, 2, DIM], F8, kind="ExternalInput")
    w1_d = nc.dram_tensor("w18i", [128, KP, MC_H, 256], F8, kind="ExternalInput")
    if fc2_fp8:
        wf2_d = nc.dram_tensor("wf28", [128, KP2, 2, DIM], F8, kind="ExternalInput")
    else:
        wf2_d = nc.dram_tensor("wf2", [HIDDEN, DIM], BF16, kind="ExternalInput")
    qkb_d = nc.dram_tensor("qkb_pm", [128, 2 * FC], F32, kind="ExternalInput")
    f1b_d = nc.dram_tensor("fc1b_pm", [128, MC_H], F32, kind="ExternalInput")
    if with_bias:
        vb_d = nc.dram_tensor("vb_row", [1, DIM], BF16, kind="ExternalInput")
        pb_d = nc.dram_tensor("projb_row", [1, DIM], BF16, kind="ExternalInput")
        f2b_d = nc.dram_tensor("fc2b_row", [1, DIM], BF16, kind="ExternalInput")
    out_d = nc.dram_tensor("out", [N_TOK, DIM], F32, kind="ExternalOutput")

    x_dt = x_d.ap().rearrange("(t p) c -> p t c", p=128)
    out_dt = out_d.ap().rearrange("(t p) c -> p t c", p=128)
    if not fc2_fp8:
        wf2_3 = wf2_d.ap().rearrange("(ko p) n -> p ko n", p=128)

    with tile.TileContext(nc) as tc:
        with (
            tc.tile_pool(name="const", bufs=1) as const_pool,
            tc.tile_pool(name="resid", bufs=1) as resid_pool,
            tc.tile_pool(name="stats", bufs=4) as stat_pool,
            tc.tile_pool(name="wsmall", bufs=1) as ws_pool,
            tc.tile_pool(name="h2fm", bufs=1) as h2_pool,
            tc.tile_pool(name="qk", bufs=1) as qk_pool,
            tc.tile_pool(name="vp", bufs=1) as v_pool,
            tc.tile_pool(name="ctxp", bufs=1) as ctx_pool,
            tc.tile_pool(name="exps", bufs=12) as e_pool,
            tc.tile_pool(name="dsmall", bufs=2) as d_pool,
            tc.tile_pool(name="outt", bufs=2) as o_pool,
            # PSUM: 2x[128,2,512] (4 banks) + 2x[128,512] (2) + 2x[128,512] (2)
            tc.tile_pool(name="psum_s", bufs=2, space="PSUM") as psum_s,
            tc.tile_pool(name="psum_sm", bufs=2, space="PSUM") as psum_sm,
            tc.tile_pool(name="psum_cp", bufs=2, space="PSUM") as psum_cp,
        ):
            # ---------------- constants + small weights ----------------
            ident = const_pool.tile([128, 128], BF16)
            make_identity(nc, ident)
            eps_t = const_pool.tile([128, 1], F32)
            nc.vector.memset(eps_t, EPS)
            qkb = const_pool.tile([128, 2 * FC], F32)
            nc.sync.dma_start(out=qkb, in_=qkb_d.ap())
            f1b = const_pool.tile([128, MC_H], F32)
            nc.sync.dma_start(out=f1b, in_=f1b_d.ap())
            if with_bias:
                ones_bf = const_pool.tile([1, 128], BF16)
                nc.vector.memset(ones_bf, 1.0)
                vb_row = const_pool.tile([1, DIM], BF16)
                nc.sync.dma_start(out=vb_row, in_=vb_d.ap())
                pb_row = const_pool.tile([1, DIM], BF16)
                nc.sync.dma_start(out=pb_row, in_=pb_d.ap())
                f2b_row = const_pool.tile([1, DIM], BF16)
                nc.sync.dma_start(out=f2b_row, in_=f2b_d.ap())
            wp = ws_pool.tile([128, KP, 2, DIM], F8, name="wp")
            nc.sync.dma_start(out=wp, in_=wp_d.ap())

            x_sb = resid_pool.tile([128, TC, DIM], F32)
            h2_fm = h2_pool.tile([128, FC, N_TOK], F8)
            qk_fm = qk_pool.tile([128, 2 * FC, N_TOK], BF16)
            ctx_fm = ctx_pool.tile([128, FC, N_TOK], F8)
            # v_pair[:, t, j, :]: [0:64] v of head 2j | [64] 1/32 (den even) |
            # [65] 1/32 (den odd) | [66:129] zeros | [129:193] v head 2j+1
            v_pair = v_pool.tile([128, TC, FC, VP], F8)
            nc.vector.memset(v_pair[:, :, :, 64:129], 0.0)
            nc.vector.memset(v_pair[:, :, :, 64:66], 1.0 / WS)

            # ---------------- helpers ----------------
            def ln_stats(t):
                st = stat_pool.tile([128, 3, 6], F32, tag="lnst")
                for sg in range(3):
                    nc.vector.bn_stats(
                        out=st[:, sg, :],
                        in_=x_sb[:, t, sg * 256:(sg + 1) * 256])
                mv = stat_pool.tile([128, 2], F32, tag="lnmv")
                nc.vector.bn_aggr(out=mv, in_=st)
                return mv

            def ln_chunk(t, h16, mv=None):
                if mv is None:
                    mv = ln_stats(t)
                lnv = stat_pool.tile([128, 1], F32, tag="lnl")
                nc.scalar.activation(
                    out=lnv, in_=mv[:, 1:2],
                    func=mybir.ActivationFunctionType.Ln, bias=eps_t)
                rstd = stat_pool.tile([128, 1], F32, tag="lnr")
                nc.scalar.activation(
                    out=rstd, in_=lnv,
                    func=mybir.ActivationFunctionType.Exp, scale=-0.5)
                nc.vector.tensor_scalar(
                    out=h16, in0=x_sb[:, t, :], scalar1=mv[:, 0:1],
                    scalar2=rstd,
                    op0=mybir.AluOpType.subtract, op1=mybir.AluOpType.mult)

            def ln_transpose(t, dst, copy_eng, mv=None):
                h16 = stat_pool.tile([128, DIM], BF16, tag="h16")
                ln_chunk(t, h16, mv)
                tr = psum_sm.tile([128, FC, 128], BF16, tag="sm")
                for f in range(FC):
                    nc.tensor.transpose(
                        tr[:, f, :], h16[:, f * 128:(f + 1) * 128], ident)
                if copy_eng == "act":
                    nc.scalar.copy(
                        out=dst[:, :, t * 128:(t + 1) * 128], in_=tr)
                else:
                    nc.vector.tensor_copy(
                        out=dst[:, :, t * 128:(t + 1) * 128], in_=tr)

            def emit_v(t, h_fm, wv):
                for nv in range(2):
                    ps = psum_sm.tile([128, 384], F32, tag="sm")
                    for kp in range(KP):
                        nc.tensor.matmul(
                            ps,
                            h_fm[:, 2 * kp:2 * kp + 2, t * 128:(t + 1) * 128],
                            wv[:, kp, :, nv * 384:(nv + 1) * 384],
                            start=(kp == 0),
                            stop=(not with_bias and kp == KP - 1),
                            perf_mode=DR)
                    if with_bias:
                        nc.tensor.matmul(
                            ps, ones_bf, vb_row[0:1, nv * 384:(nv + 1) * 384],
                            start=False, stop=True)
                    pshd = ps.rearrange("p (h d) -> p h d", d=HD)
                    jsl = slice(nv * 3, nv * 3 + 3)
                    nc.vector.tensor_scalar(
                        out=v_pair[:, t, jsl, 0:HD], in0=pshd[:, 0::2, :],
                        scalar1=1.0 / WS, scalar2=None,
                        op0=mybir.AluOpType.mult)
                    nc.vector.tensor_scalar(
                        out=v_pair[:, t, jsl, 129:193], in0=pshd[:, 1::2, :],
                        scalar1=1.0 / WS, scalar2=None,
                        op0=mybir.AluOpType.mult)

            def emit_qk(m, h_fm, wqk):
                """qk_fm[:, m, :] for feature chunk m (q: m<6, k: m>=6)."""
                for q in range(2):
                    ps = psum_sm.tile([128, 512], F32, tag="sm")
                    for kp in range(KP):
                        nc.tensor.matmul(
                            ps,
                            wqk[:, kp, m, :].rearrange(
                                "p (two c) -> p two c", two=2),
                            h_fm[:, 2 * kp:2 * kp + 2, q * 512:(q + 1) * 512],
                            start=(kp == 0), stop=(kp == KP - 1),
                            perf_mode=DRI)
                    nc.vector.tensor_scalar(
                        out=qk_fm[:, m, q * 512:(q + 1) * 512], in0=ps,
                        scalar1=1.0 / WS, scalar2=qkb[:, m:m + 1],
                        op0=mybir.AluOpType.mult, op1=mybir.AluOpType.add)

            def emit_s_exp(j, half):
                """S + exp for head pair j, q-half; returns E[kcp][ab].

                The two heads' stationaries live at partitions 0:64 / 64:128
                (PE row groups h0/h64); interleaving ab inside the i loop lets
                the hardware run the two 64-deep matmuls concurrently."""
                exps = []
                for kcp in range(4):
                    sp = [psum_s.tile([128, 2, 512], F32, tag="s",
                                      name=f"sp_{ab}")
                          for ab in range(2)]
                    for i in range(2):
                        kc = 2 * kcp + i
                        for ab in range(2):
                            po = 64 * ab
                            nc.tensor.matmul(
                                sp[ab][:, i, :],
                                qk_fm[po:po + 64, 6 + j,
                                      kc * 128:(kc + 1) * 128],
                                qk_fm[po:po + 64, j,
                                      half * 512:(half + 1) * 512],
                                start=True, stop=True)
                    pair = []
                    for ab in range(2):
                        e_t = e_pool.tile([128, 2, 512], F8, tag="e")
                        nc.scalar.activation(
                            out=e_t, in_=sp[ab],
                            func=mybir.ActivationFunctionType.Exp, scale=SCALE)
                        pair.append(e_t)
                    exps.append(pair)
                return exps

            def emit_ctx(j, half, exps):
                qsl = slice(half * 512, (half + 1) * 512)
                for ab in range(2):
                    vsl = slice(0, 65) if ab == 0 else slice(65, 193)
                    cp = psum_cp.tile([128, 512], F32, tag="cp")
                    cpv = cp[0:65] if ab == 0 else cp
                    for kcp in range(4):
                        nc.tensor.matmul(
                            cpv, v_pair[:, 2 * kcp:2 * kcp + 2, j, vsl],
                            exps[kcp][ab],
                            start=(kcp == 0), stop=(kcp == 3), perf_mode=DR)
                    den = cp[64:65, :] if ab == 0 else cp[0:1, :]
                    dcp = d_pool.tile([1, 512], F32, tag="dcp")
                    nc.vector.tensor_copy(out=dcp, in_=den)
                    rec = d_pool.tile([1, 512], F32, tag="rec")
                    nc.vector.reciprocal_approx_fast(out=rec, in_=dcp)
                    bcd = d_pool.tile([128, 512], F32, tag="bcd")
                    nc.gpsimd.partition_broadcast(bcd, rec)
                    po = 64 * ab
                    nc.vector.tensor_tensor(
                        out=ctx_fm[po:po + 64, j, qsl],
                        in0=cp[po:po + 64, :], in1=bcd[po:po + 64, :],
                        op=mybir.AluOpType.mult)

            def emit_proj(t):
                for nv in range(2):
                    ps = psum_sm.tile([128, 384], F32, tag="sm")
                    for kp in range(KP):
                        nc.tensor.matmul(
                            ps, ctx_fm[:, 2 * kp:2 * kp + 2,
                                       t * 128:(t + 1) * 128],
                            wp[:, kp, :, nv * 384:(nv + 1) * 384],
                            start=(kp == 0),
                            stop=(not with_bias and kp == KP - 1),
                            perf_mode=DR)
                    if with_bias:
                        nc.tensor.matmul(
                            ps, ones_bf, pb_row[0:1, nv * 384:(nv + 1) * 384],
                            start=False, stop=True)
                    sl = slice(nv * 384, (nv + 1) * 384)
                    pa = o_pool.tile([128, 384], F32, tag="pa")
                    nc.vector.tensor_scalar(
                        out=pa, in0=ps, scalar1=1.0 / (WS * WS), scalar2=None,
                        op0=mybir.AluOpType.mult)
                    nc.gpsimd.tensor_tensor(
                        out=x_sb[:, t, sl], in0=pa, in1=x_sb[:, t, sl],
                        op=mybir.AluOpType.add)

            def emit_fc1(m, half, w1):
                ps = psum_s.tile([128, 512], F32, tag="s")
                for kp in range(KP):
                    nc.tensor.matmul(
                        ps,
                        w1[:, kp, m, :].rearrange(
                            "p (two c) -> p two c", two=2),
                        h2_fm[:, 2 * kp:2 * kp + 2,
                              half * 512:(half + 1) * 512],
                        start=(kp == 0), stop=(kp == KP - 1),
                        perf_mode=DRI)
                return ps

            def emit_gelu(m, half, ps, g_fm):
                # g_fm is half-buffered [128, MC_H, 512]: half 1 reuses the
                # storage of half 0 (fc2 of half 0 is emitted in between).
                nc.scalar.activation(
                    out=g_fm[:, m, :], in_=ps,
                    func=mybir.ActivationFunctionType.Gelu,
                    bias=f1b[:, m:m + 1], scale=1.0 / WS)

            def emit_fc2(t, g_fm, wf2):
                th = (t % 4) * 128
                for nv in range(2):
                    ps = psum_sm.tile([128, 384], F32, tag="sm")
                    if fc2_fp8:
                        for kp in range(KP2):
                            nc.tensor.matmul(
                                ps,
                                g_fm[:, 2 * kp:2 * kp + 2, th:th + 128],
                                wf2[:, kp, :, nv * 384:(nv + 1) * 384],
                                start=(kp == 0),
                                stop=(not with_bias and kp == KP2 - 1),
                                perf_mode=DR)
                    else:
                        for k in range(MC_H):
                            nc.tensor.matmul(
                                ps, g_fm[:, k, th:th + 128],
                                wf2[nv][:, k, :],
                                start=(k == 0),
                                stop=(not with_bias and k == MC_H - 1))
                    if with_bias:
                        nc.tensor.matmul(
                            ps, ones_bf, f2b_row[0:1, nv * 384:(nv + 1) * 384],
                            start=False, stop=True)
                    sl = slice(nv * 384, (nv + 1) * 384)
                    o_t = o_pool.tile([128, 384], F32, tag="ot")
                    if fc2_fp8:
                        nc.vector.tensor_scalar(
                            out=o_t, in0=ps, scalar1=1.0 / WS, scalar2=None,
                            op0=mybir.AluOpType.mult)
                        nc.vector.tensor_tensor(
                            out=o_t, in0=o_t, in1=x_sb[:, t, sl],
                            op=mybir.AluOpType.add)
                    else:
                        nc.vector.tensor_add(
                            out=o_t, in0=ps, in1=x_sb[:, t, sl])
                    nc.sync.dma_start(out=out_dt[:, t, sl], in_=o_t)

            # ---------------- emission ----------------
            with (
                tc.tile_pool(name="hfm", bufs=1) as hfm_pool,
                tc.tile_pool(name="wbig", bufs=1) as wb_pool,
            ):
                wqk = wb_pool.tile([128, KP, 2 * FC, 256], F8, name="wqk")
                wv = wb_pool.tile([128, KP, 2, DIM], F8, name="wv")
                h_fm = hfm_pool.tile([128, FC, N_TOK], F8)

                for t in range(TC):
                    nc.sync.dma_start(out=x_sb[:, t, :], in_=x_dt[:, t, :])
                    ln_transpose(t, h_fm, "act")
                    if t == 0:
                        nc.sync.dma_start(out=wv, in_=wv_d.ap())
                    if t == 1:
                        nc.sync.dma_start(out=wqk, in_=wqk_d.ap())
                emit_qk(6, h_fm, wqk)
                emit_qk(0, h_fm, wqk)

                # attention half 0 (v + remaining q/k streamed into rounds)
                pend = None
                for j in range(6):
                    if pend is not None:
                        emit_ctx(pend[0], 0, pend[1])
                    exps = emit_s_exp(j, 0)
                    pend = (j, exps)
                    if j == 0:
                        for t in range(TC):
                            emit_v(t, h_fm, wv)
                    if j < 5:
                        emit_qk(7 + j, h_fm, wqk)
                        emit_qk(1 + j, h_fm, wqk)
                emit_ctx(pend[0], 0, pend[1])

            with (
                tc.tile_pool(name="gfm", bufs=1) as g_pool,
                tc.tile_pool(name="wmlp", bufs=1) as wm_pool,
            ):
                w1 = wm_pool.tile([128, KP, MC_H, 256], F8, name="w1")
                nc.sync.dma_start(out=w1, in_=w1_d.ap())
                if fc2_fp8:
                    wf2 = wm_pool.tile([128, KP2, 2, DIM], F8, name="wf28")
                    nc.sync.dma_start(out=wf2, in_=wf2_d.ap())
                    g_fm = g_pool.tile([128, MC_H, 512], F8)
                else:
                    wf2 = [wm_pool.tile([128, MC_H, 384], BF16, name=f"wf2{i}")
                           for i in range(2)]
                    for nv in range(2):
                        nc.sync.dma_start(
                            out=wf2[nv],
                            in_=wf2_3[:, :, nv * 384:(nv + 1) * 384])
                    g_fm = g_pool.tile([128, MC_H, 512], BF16)

                # attention half 1, proj/LN2 of half 0 interleaved
                mv2 = {}
                pend = None
                for j in range(6):
                    if pend is not None:
                        emit_ctx(pend[0], 1, pend[1])
                    exps = emit_s_exp(j, 1)
                    pend = (j, exps)
                    if j >= 2:
                        emit_proj(j - 2)
                        mv2[j - 2] = ln_stats(j - 2)
                emit_ctx(pend[0], 1, pend[1])
                for t in range(4):
                    ln_transpose(t, h2_fm, "vec", mv2[t])
                    emit_proj(t + 4)
                for t in range(4, 8):
                    ln_transpose(t, h2_fm, "vec")

                # MLP (single table switch to gelu before the first fc1)
                for m in range(MC_H):
                    ps = emit_fc1(m, 0, w1)
                    emit_gelu(m, 0, ps, g_fm)
                for t in range(4):
                    emit_fc2(t, g_fm, wf2)
                for m in range(MC_H):
                    ps = emit_fc1(m, 1, w1)
                    emit_gelu(m, 1, ps, g_fm)
                for t in range(4, 8):
                    emit_fc2(t, g_fm, wf2)

    nc.compile()
    return nc


FC2_FP8 = False


def host_prep(x, ln1_g, ln1_b, qkv_w, proj_w, proj_b, ln2_g, ln2_b,
              fc1_w, fc1_b, fc2_w, fc2_b, fc2_fp8=FC2_FP8):
    """Fold LN affines into weights, quantize to fp8e4 (x32) / bf16."""
    import ml_dtypes
    f32 = np.float32
    bf16 = ml_dtypes.bfloat16
    f8 = ml_dtypes.float8_e4m3  # TRN e4m3: bias 7, max 240

    def q8(a):
        return np.ascontiguousarray(
            np.clip(a * WS, -240.0, 240.0).astype(f8))

    def drswi_pack(wt):
        # [768, n] -> [128, KP, n//128, 256]: il[p,kp,m,2k+i] =
        # wt[(2kp+i)*128+p, m*128+127-k] (A/B interleaved, cols reversed)
        n = wt.shape[1]
        a = wt.reshape(KP, 2, 128, n // 128, 128)[:, :, :, :, ::-1]
        return np.ascontiguousarray(
            a.transpose(2, 0, 3, 4, 1).reshape(128, KP, n // 128, 256))

    def dr_pack(wt):
        # [kin, n out] -> [128, kin//256, 2, n], in-feature = (2kp+i)*128+p
        kin, n = wt.shape
        return np.ascontiguousarray(
            wt.reshape(kin // 256, 2, 128, n).transpose(2, 0, 1, 3))

    qkv_w = np.asarray(qkv_w, f32)
    qkv_wt = (qkv_w * np.asarray(ln1_g, f32)[None, :]).T  # [768, 2304]
    qkv_bias = qkv_w @ np.asarray(ln1_b, f32)
    wqk8 = q8(drswi_pack(qkv_wt[:, :2 * DIM]))
    wv8 = q8(dr_pack(qkv_wt[:, 2 * DIM:]))
    qkb_pm = np.ascontiguousarray(qkv_bias[:2 * DIM].reshape(2 * FC, 128).T)
    vb_row = np.ascontiguousarray(
        (WS * qkv_bias[2 * DIM:]).astype(bf16).reshape(1, DIM))

    proj_wt = np.ascontiguousarray(np.asarray(proj_w, f32).T)
    wp8 = q8(dr_pack(proj_wt))
    projb_row = np.ascontiguousarray(
        (WS * WS * np.asarray(proj_b, f32)).astype(bf16).reshape(1, DIM))

    fc1_w = np.asarray(fc1_w, f32)
    fc1_wt = (fc1_w * np.asarray(ln2_g, f32)[None, :]).T  # [768, 3072]
    w18 = q8(drswi_pack(fc1_wt))
    fc1_bias = fc1_w @ np.asarray(ln2_b, f32) + np.asarray(fc1_b, f32)
    fc1b_pm = np.ascontiguousarray(fc1_bias.reshape(MC_H, 128).T)

    fc2_wt = np.ascontiguousarray(np.asarray(fc2_w, f32).T)  # [3072, 768]
    fc2b_row = np.ascontiguousarray(
        np.asarray(fc2_b, f32).astype(bf16).reshape(1, DIM))

    w = {
        "wqk8i": wqk8, "wv8": wv8, "wp8": wp8, "w18i": w18,
        "qkb_pm": qkb_pm, "fc1b_pm": fc1b_pm,
    }
    if fc2_fp8:
        w["wf28"] = q8(dr_pack(fc2_wt))
    else:
        w["wf2"] = np.ascontiguousarray(fc2_wt.astype(bf16))

    has_bias = not (
        np.all(qkv_bias[2 * DIM:] == 0.0)
        and np.all(np.asarray(proj_b, f32) == 0.0)
        and np.all(np.asarray(fc2_b, f32) == 0.0))
    if has_bias:
        w["vb_row"] = vb_row
        w["projb_row"] = projb_row
        w["fc2b_row"] = fc2b_row
    return w, has_bias


_CACHE = {}


def kernel(x, ln1_g, ln1_b, qkv_w, proj_w, proj_b, ln2_g, ln2_b,
           fc1_w, fc1_b, fc2_w, fc2_b, _want_results=False, **_ignored):
    from concourse.bass_utils import run_bass_kernel_spmd

    x = np.asarray(x, np.float32)
    B = x.shape[0]
    assert B == 8 and x.shape[1] == N_TOK and x.shape[2] == DIM

    w, has_bias = host_prep(x, ln1_g, ln1_b, qkv_w, proj_w, proj_b, ln2_g,
                            ln2_b, fc1_w, fc1_b, fc2_w, fc2_b)

    key = ("nc", has_bias, FC2_FP8)
    if key not in _CACHE:
        _CACHE[key] = build_bass(with_bias=has_bias, fc2_fp8=FC2_FP8)
        _CACHE["nc"] = _CACHE[key]
    nc = _CACHE[key]

    in_maps = [dict(w, x=np.ascontiguousarray(x[i])) for i in range(B)]
    res = run_bass_kernel_spmd(nc, in_maps, core_ids=list(range(B)))
    out = np.stack([res.results[i]["out"] for i in range(B)], axis=0)
    if _want_results:
        return out, res
    return out


# revision 17
# speedup vs baseline: 1.1935x; 1.1935x over previous
"""Trainium2 Bass kernel for a dense transformer block (pre-LN, 12 heads, MLP 4x).

Strategy: data-parallel over batch across the 8 NeuronCores (B=8 -> one batch
element per core, no collectives). Per core:

  - residual stream token-major f32 [128 tok x 8 chunks x 768] (DMA'd straight
    from HBM, no cast)
  - LN on DVE (bn_stats/bn_aggr); rstd = Exp(-0.5*Ln(var+eps)) on ACT so the
    whole kernel needs only the natural_log_exp table set plus one gelu load
  - fp8e4 (TRN e4m3, max 240) DoubleRow matmuls for qkv, ctx (E@V), proj and
    fc1: weights host-quantized at 32x scale, activations at unit scale;
    descales fold into the consuming DVE/ACT op. S (q@k) stays bf16; fc2 is
    fp8-DR or bf16 depending on the error budget (fc2_fp8 flag).
  - S (q@k, contraction = head_dim 64) uses PE row tiling: the two heads of a
    pair sit at partitions 0:64 / 64:128, so their matmuls land in row groups
    h0/h64 and run CONCURRENTLY in the PE array when interleaved - that is the
    point of the (i, ab) emission order in emit_s_exp.
  - softmax: S psum tiles [128, 2kc, 512q]; one exp per tile writing fp8 in
    the pair layout the DoubleRow ctx matmul consumes. Denominator via a
    1/32-scaled ones column in the stationary V operand; odd heads of a pair
    put v in cols 129..192 so their ctx lands on partitions 64..127 with the
    denominator on partition 0 - no cross-partition fixups. Normalization:
    reciprocal_approx_fast + gpsimd partition_broadcast + one DVE mult.
  - attention is processed in two q-halves so ctx/proj of half 0 overlap the
    exp stream of half 1. fc1+gelu+fc2 sit after the last exp because the
    gelu table set must not interleave with the exp set (2.7us reload per
    flap) and fc1 psum tiles serialize against their gelu.
  - bias matmuls (ones-row trick) are only emitted when the corresponding
    biases are nonzero; for this problem's inputs they are all zero.
"""

import numpy as np

import concourse.bass as bass
import concourse.mybir as mybir
import concourse.tile as tile
from concourse import bacc
from concourse.masks import make_identity

DIM = 768
HEADS = 12
HD = 64
HIDDEN = 3072
N_TOK = 1024
TC = N_TOK // 128  # 8 token chunks
FC = DIM // 128  # 6 feature chunks
KP = 3  # DoubleRow contraction pairs over 768
KP2 = HIDDEN // 256  # 12 DoubleRow pairs over 3072
MC_H = HIDDEN // 128  # 24 hidden chunks
EPS = 1e-5
SCALE = HD ** -0.5
WS = 32.0  # fp8 weight upscale
VP = 208  # per-head-pair free-dim stride in v_pair

F32 = mybir.dt.float32
BF16 = mybir.dt.bfloat16
F8 = mybir.dt.float8e4
DR = mybir.MatmulPerfMode.DoubleRow
DRI = mybir.MatmulPerfMode.DoubleRowSwInterleave


def build_bass(with_bias=True, fc2_fp8=False):
    nc = bacc.Bacc("TRN2", debug=False)

    x_d = nc.dram_tensor("x", [N_TOK, DIM], F32, kind="ExternalInput")
    wqk_d = nc.dram_tensor("wqk8i", [128, KP, 2 * FC, 256], F8, kind="ExternalInput")
    wv_d = nc.dram_tensor("wv8", [128, KP, 2, DIM], F8, kind="ExternalInput")
    wp_d = nc.dram_tensor("wp8", [128, KP, 2, DIM], F8, kind="ExternalInput")
    w1_d = nc.dram_tensor("w18i", [128, KP, MC_H, 256], F8, kind="ExternalInput")
    if fc2_fp8:
        wf2_d = nc.dram_tensor("wf28", [128, KP2, 2, DIM], F8, kind="ExternalInput")
    else:
        wf2_d = nc.dram_tensor("wf2", [HIDDEN, DIM], BF16, kind="ExternalInput")
    qkb_d = nc.dram_tensor("qkb_pm", [128, 2 * FC], F32, kind="ExternalInput")
    f1b_d = nc.dram_tensor("fc1b_pm", [128, MC_H], F32, kind="ExternalInput")
    if with_bias:
        vb_d = nc.dram_tensor("vb_row", [1, DIM], BF16, kind="ExternalInput")
        pb_d = nc.dram_tensor("projb_row", [1, DIM], BF16, kind="ExternalInput")
        f2b_d = nc.dram_tensor("fc2b_row", [1, DIM], BF16, kind="ExternalInput")
    out_d = nc.dram_tensor("out", [N_TOK, DIM], F32, kind="ExternalOutput")

    x_dt = x_d.ap().rearrange("(t p) c -> p t c", p=128)
    out_dt = out_d.ap().rearrange("(t p) c -> p t c", p=128)
    if not fc2_fp8:
        wf2_3 = wf2_d.ap().rearrange("(ko p) n -> p ko n", p=128)

    with tile.TileContext(nc) as tc:
        with (
            tc.tile_pool(name="const", bufs=1) as const_pool,
            tc.tile_pool(name="resid", bufs=1) as resid_pool,
            tc.tile_pool(name="stats", bufs=4) as stat_pool,
            tc.tile_pool(name="wsmall", bufs=1) as ws_pool,
            tc.tile_pool(name="h2fm", bufs=1) as h2_pool,
            tc.tile_pool(name="qk", bufs=1) as qk_pool,
            tc.tile_pool(name="vp", bufs=1) as v_pool,
            tc.tile_pool(name="ctxp", bufs=1) as ctx_pool,
            tc.tile_pool(name="exps", bufs=1) as e_pool,
            tc.tile_pool(name="dsmall", bufs=2) as d_pool,
            tc.tile_pool(name="outt", bufs=2) as o_pool,
            # PSUM: 2x[128,2,512] (4 banks) + 2x[128,512] (2) + 2x[128,512] (2)
            tc.tile_pool(name="psum_s", bufs=2, space="PSUM") as psum_s,
            tc.tile_pool(name="psum_sm", bufs=2, space="PSUM") as psum_sm,
            tc.tile_pool(name="psum_cp", bufs=2, space="PSUM") as psum_cp,
        ):
            # ---------------- constants + small weights ----------------
            ident = const_pool.tile([128, 128], BF16)
            make_identity(nc, ident)
            magic_u = const_pool.tile([128, 1], F32)
            nc.vector.memset(magic_u.bitcast(mybir.dt.uint32), 0x5F3759DF)
            qkb = const_pool.tile([128, 2 * FC], F32)
            nc.sync.dma_start(out=qkb, in_=qkb_d.ap())
            f1b = const_pool.tile([128, MC_H], F32)
            nc.sync.dma_start(out=f1b, in_=f1b_d.ap())
            if with_bias:
                ones_bf = const_pool.tile([1, 128], BF16)
                nc.vector.memset(ones_bf, 1.0)
                vb_row = const_pool.tile([1, DIM], BF16)
                nc.sync.dma_start(out=vb_row, in_=vb_d.ap())
                pb_row = const_pool.tile([1, DIM], BF16)
                nc.sync.dma_start(out=pb_row, in_=pb_d.ap())
                f2b_row = const_pool.tile([1, DIM], BF16)
                nc.sync.dma_start(out=f2b_row, in_=f2b_d.ap())
            wp = ws_pool.tile([128, KP, 2, DIM], F8, name="wp")
            nc.sync.dma_start(out=wp, in_=wp_d.ap())

            x_sb = resid_pool.tile([128, TC, DIM], F32)
            h2_fm = h2_pool.tile([128, FC, N_TOK], F8)
            qk_fm = qk_pool.tile([128, 2 * FC, N_TOK], BF16)
            ctx_fm = ctx_pool.tile([128, FC, N_TOK], F8)
            # v_pair[:, t, j, :]: [0:64] v of head 2j | [64] 1/32 (den even) |
            # [65] 1/32 (den odd) | [66:129] zeros | [129:193] v head 2j+1
            v_pair = v_pool.tile([128, TC, FC, VP], F8)
            nc.vector.memset(v_pair[:, :, :, 64:129], 0.0)
            nc.vector.memset(v_pair[:, :, :, 64:66], 1.0 / WS)
            # exp staging: [p, ab, phase, kc, q]; consecutive attention
            # rounds alternate phase so ctx(j-1) reads one phase while the
            # S/exp stream of round j fills the other.
            e_buf = e_pool.tile([128, 2, 2, 8, 512], F8)

            # ---------------- helpers ----------------
            def ln_stats(t):
                st = stat_pool.tile([128, 3, 6], F32, tag="lnst")
                for sg in range(3):
                    nc.vector.bn_stats(
                        out=st[:, sg, :],
                        in_=x_sb[:, t, sg * 256:(sg + 1) * 256])
                mv = stat_pool.tile([128, 2], F32, tag="lnmv")
                nc.vector.bn_aggr(out=mv, in_=st)
                return mv

            def ln_chunk(t, h16, mv=None, h16_eng="vec"):
                # rstd = rsqrt(var+eps) entirely on DVE (magic-number seed +
                # one Newton step, ~0.1% rel err, far below the fp8 noise) so
                # the ACT engine never needs the sqrt/ln table sets.
                if mv is None:
                    mv = ln_stats(t)
                vh = stat_pool.tile([128, 1], F32, tag="lnv")
                nc.vector.tensor_scalar(
                    out=vh, in0=mv[:, 1:2], scalar1=EPS, scalar2=None,
                    op0=mybir.AluOpType.add)
                nh = stat_pool.tile([128, 1], F32, tag="lnn")
                nc.vector.tensor_scalar(
                    out=nh, in0=vh, scalar1=-0.5, scalar2=None,
                    op0=mybir.AluOpType.mult)
                y = stat_pool.tile([128, 1], F32, tag="lnr")
                nc.vector.tensor_scalar(
                    out=y.bitcast(mybir.dt.uint32),
                    in0=vh.bitcast(mybir.dt.uint32), scalar1=1, scalar2=None,
                    op0=mybir.AluOpType.logical_shift_right)
                nc.vector.tensor_tensor(
                    out=y.bitcast(mybir.dt.uint32),
                    in0=magic_u.bitcast(mybir.dt.uint32),
                    in1=y.bitcast(mybir.dt.uint32),
                    op=mybir.AluOpType.subtract)
                b = stat_pool.tile([128, 1], F32, tag="lnb")
                nc.vector.tensor_tensor(
                    out=b, in0=y, in1=y, op=mybir.AluOpType.mult)
                nc.vector.tensor_scalar(
                    out=b, in0=b, scalar1=nh, scalar2=1.5,
                    op0=mybir.AluOpType.mult, op1=mybir.AluOpType.add)
                rstd = stat_pool.tile([128, 1], F32, tag="lnr2")
                nc.vector.tensor_tensor(
                    out=rstd, in0=y, in1=b, op=mybir.AluOpType.mult)
                eng = nc.gpsimd if h16_eng == "gps" else nc.vector
                eng.tensor_scalar(
                    out=h16, in0=x_sb[:, t, :], scalar1=mv[:, 0:1],
                    scalar2=rstd,
                    op0=mybir.AluOpType.subtract, op1=mybir.AluOpType.mult)

            def ln_transpose(t, dst, copy_eng, mv=None, h16_eng="vec"):
                h16 = stat_pool.tile([128, DIM], BF16, tag="h16")
                ln_chunk(t, h16, mv, h16_eng)
                tr = psum_sm.tile([128, FC, 128], BF16, tag="sm")
                for f in range(FC):
                    nc.tensor.transpose(
                        tr[:, f, :], h16[:, f * 128:(f + 1) * 128], ident)
                if copy_eng == "act":
                    nc.scalar.copy(
                        out=dst[:, :, t * 128:(t + 1) * 128], in_=tr)
                else:
                    nc.vector.tensor_copy(
                        out=dst[:, :, t * 128:(t + 1) * 128], in_=tr)

            def emit_v(t, h_fm, wv):
                for nv in range(2):
                    ps = psum_sm.tile([128, 384], F32, tag="sm")
                    for kp in range(KP):
                        nc.tensor.matmul(
                            ps,
                            h_fm[:, 2 * kp:2 * kp + 2, t * 128:(t + 1) * 128],
                            wv[:, kp, :, nv * 384:(nv + 1) * 384],
                            start=(kp == 0),
                            stop=(not with_bias and kp == KP - 1),
                            perf_mode=DR)
                    if with_bias:
                        nc.tensor.matmul(
                            ps, ones_bf, vb_row[0:1, nv * 384:(nv + 1) * 384],
                            start=False, stop=True)
                    pshd = ps.rearrange("p (h d) -> p h d", d=HD)
                    jsl = slice(nv * 3, nv * 3 + 3)
                    nc.vector.tensor_scalar(
                        out=v_pair[:, t, jsl, 0:HD], in0=pshd[:, 0::2, :],
                        scalar1=1.0 / WS, scalar2=None,
                        op0=mybir.AluOpType.mult)
                    nc.vector.tensor_scalar(
                        out=v_pair[:, t, jsl, 129:193], in0=pshd[:, 1::2, :],
                        scalar1=1.0 / WS, scalar2=None,
                        op0=mybir.AluOpType.mult)

            def emit_qk(m, h_fm, wqk):
                """qk_fm[:, m, :] for feature chunk m (q: m<6, k: m>=6)."""
                for q in range(2):
                    ps = psum_sm.tile([128, 512], F32, tag="sm")
                    for kp in range(KP):
                        nc.tensor.matmul(
                            ps,
                            wqk[:, kp, m, :].rearrange(
                                "p (two c) -> p two c", two=2),
                            h_fm[:, 2 * kp:2 * kp + 2, q * 512:(q + 1) * 512],
                            start=(kp == 0), stop=(kp == KP - 1),
                            perf_mode=DRI)
                    nc.vector.tensor_scalar(
                        out=qk_fm[:, m, q * 512:(q + 1) * 512], in0=ps,
                        scalar1=1.0 / WS, scalar2=qkb[:, m:m + 1],
                        op0=mybir.AluOpType.mult, op1=mybir.AluOpType.add)

            def emit_s_exp(j, half):
                """S + exp for head pair j, q-half; returns the e_buf phase.

                The two heads' stationaries live at partitions 0:64 / 64:128
                (PE row groups h0/h64), so the two matmuls of one kc chunk run
                CONCURRENTLY in the PE array. Both land in one psum tile so
                the pair stays together through the scheduler, and one exp
                drains both."""
                ph = j % 2
                for kc in range(8):
                    sp = psum_s.tile([128, 2, 512], F32, tag="s", name="sp")
                    for ab in range(2):
                        po = 64 * ab
                        nc.tensor.matmul(
                            sp[:, ab, :],
                            qk_fm[po:po + 64, 6 + j,
                                  kc * 128:(kc + 1) * 128],
                            qk_fm[po:po + 64, j,
                                  half * 512:(half + 1) * 512],
                            start=True, stop=True)
                    nc.scalar.activation(
                        out=e_buf[:, :, ph, kc, :], in_=sp,
                        func=mybir.ActivationFunctionType.Exp, scale=SCALE)
                return ph

            def emit_ctx(j, half, ph):
                qsl = slice(half * 512, (half + 1) * 512)
                for ab in range(2):
                    vsl = slice(0, 65) if ab == 0 else slice(65, 193)
                    cp = psum_cp.tile([128, 512], F32, tag="cp")
                    cpv = cp[0:65] if ab == 0 else cp
                    for kcp in range(4):
                        nc.tensor.matmul(
                            cpv, v_pair[:, 2 * kcp:2 * kcp + 2, j, vsl],
                            e_buf[:, ab, ph, 2 * kcp:2 * kcp + 2, :],
                            start=(kcp == 0), stop=(kcp == 3), perf_mode=DR)
                    den = cp[64:65, :] if ab == 0 else cp[0:1, :]
                    dcp = d_pool.tile([1, 512], F32, tag="dcp")
                    nc.vector.tensor_copy(out=dcp, in_=den)
                    rec = d_pool.tile([1, 512], F32, tag="rec")
                    nc.vector.reciprocal_approx_fast(out=rec, in_=dcp)
                    bcd = d_pool.tile([128, 512], F32, tag="bcd")
                    nc.gpsimd.partition_broadcast(bcd, rec)
                    po = 64 * ab
                    nc.vector.tensor_tensor(
                        out=ctx_fm[po:po + 64, j, qsl],
                        in0=cp[po:po + 64, :], in1=bcd[po:po + 64, :],
                        op=mybir.AluOpType.mult)

            def emit_proj(t):
                for nv in range(2):
                    ps = psum_sm.tile([128, 384], F32, tag="sm")
                    for kp in range(KP):
                        nc.tensor.matmul(
                            ps, ctx_fm[:, 2 * kp:2 * kp + 2,
                                       t * 128:(t + 1) * 128],
                            wp[:, kp, :, nv * 384:(nv + 1) * 384],
                            start=(kp == 0),
                            stop=(not with_bias and kp == KP - 1),
                            perf_mode=DR)
                    if with_bias:
                        nc.tensor.matmul(
                            ps, ones_bf, pb_row[0:1, nv * 384:(nv + 1) * 384],
                            start=False, stop=True)
                    sl = slice(nv * 384, (nv + 1) * 384)
                    pa = o_pool.tile([128, 384], F32, tag="pa")
                    nc.vector.tensor_scalar(
                        out=pa, in0=ps, scalar1=1.0 / (WS * WS), scalar2=None,
                        op0=mybir.AluOpType.mult)
                    # residual add on DVE: gpsimd must stay on the
                    # partition_broadcast microcode lib (a lib swap is ~7.5us)
                    nc.vector.tensor_tensor(
                        out=x_sb[:, t, sl], in0=pa, in1=x_sb[:, t, sl],
                        op=mybir.AluOpType.add)

            def emit_fc1(m, half, w1):
                ps = psum_s.tile([128, 512], F32, tag="s")
                for kp in range(KP):
                    nc.tensor.matmul(
                        ps,
                        w1[:, kp, m, :].rearrange(
                            "p (two c) -> p two c", two=2),
                        h2_fm[:, 2 * kp:2 * kp + 2,
                              half * 512:(half + 1) * 512],
                        start=(kp == 0), stop=(kp == KP - 1),
                        perf_mode=DRI)
                return ps

            def emit_gelu(m, half, ps, g_fm):
                # g_fm is half-buffered [128, MC_H, 512]: half 1 reuses the
                # storage of half 0 (fc2 of half 0 is emitted in between).
                nc.scalar.activation(
                    out=g_fm[:, m, :], in_=ps,
                    func=mybir.ActivationFunctionType.Gelu,
                    bias=f1b[:, m:m + 1], scale=1.0 / WS)

            def emit_fc2(t, g_fm, wf2):
                th = (t % 4) * 128
                for nv in range(2):
                    ps = psum_sm.tile([128, 384], F32, tag="sm")
                    if fc2_fp8:
                        for kp in range(KP2):
                            nc.tensor.matmul(
                                ps,
                                g_fm[:, 2 * kp:2 * kp + 2, th:th + 128],
                                wf2[:, kp, :, nv * 384:(nv + 1) * 384],
                                start=(kp == 0),
                                stop=(not with_bias and kp == KP2 - 1),
                                perf_mode=DR)
                    else:
                        for k in range(MC_H):
                            nc.tensor.matmul(
                                ps, g_fm[:, k, th:th + 128],
                                wf2[nv][:, k, :],
                                start=(k == 0),
                                stop=(not with_bias and k == MC_H - 1))
                    if with_bias:
                        nc.tensor.matmul(
                            ps, ones_bf, f2b_row[0:1, nv * 384:(nv + 1) * 384],
                            start=False, stop=True)
                    sl = slice(nv * 384, (nv + 1) * 384)
                    o_t = o_pool.tile([128, 384], F32, tag="ot")
                    if fc2_fp8:
                        nc.vector.tensor_scalar(
                            out=o_t, in0=ps, scalar1=1.0 / WS, scalar2=None,
                            op0=mybir.AluOpType.mult)
                        nc.vector.tensor_tensor(
                            out=o_t, in0=o_t, in1=x_sb[:, t, sl],
                            op=mybir.AluOpType.add)
                    else:
                        nc.vector.tensor_add(
                            out=o_t, in0=ps, in1=x_sb[:, t, sl])
                    nc.sync.dma_start(out=out_dt[:, t, sl], in_=o_t)

            # ---------------- emission ----------------
            with (
                tc.tile_pool(name="hfm", bufs=1) as hfm_pool,
                tc.tile_pool(name="wbig", bufs=1) as wb_pool,
            ):
                wqk = wb_pool.tile([128, KP, 2 * FC, 256], F8, name="wqk")
                wv = wb_pool.tile([128, KP, 2, DIM], F8, name="wv")
                h_fm = hfm_pool.tile([128, FC, N_TOK], F8)

                for t in range(TC):
                    nc.sync.dma_start(out=x_sb[:, t, :], in_=x_dt[:, t, :])
                    ln_transpose(t, h_fm, "act", h16_eng="gps")
                    if t == 0:
                        nc.sync.dma_start(out=wv, in_=wv_d.ap())
                    if t == 1:
                        nc.sync.dma_start(out=wqk, in_=wqk_d.ap())
                emit_qk(6, h_fm, wqk)
                emit_qk(0, h_fm, wqk)

                # attention half 0 (v + remaining q/k streamed into rounds)
                pend = None
                for j in range(6):
                    if pend is not None:
                        emit_ctx(pend[0], 0, pend[1])
                    ph = emit_s_exp(j, 0)
                    pend = (j, ph)
                    if j == 0:
                        for t in range(TC):
                            emit_v(t, h_fm, wv)
                    if j < 5:
                        emit_qk(7 + j, h_fm, wqk)
                        emit_qk(1 + j, h_fm, wqk)
                emit_ctx(pend[0], 0, pend[1])

            with (
                tc.tile_pool(name="gfm", bufs=1) as g_pool,
                tc.tile_pool(name="wmlp", bufs=1) as wm_pool,
            ):
                w1 = wm_pool.tile([128, KP, MC_H, 256], F8, name="w1")
                nc.sync.dma_start(out=w1, in_=w1_d.ap())
                if fc2_fp8:
                    wf2 = wm_pool.tile([128, KP2, 2, DIM], F8, name="wf28")
                    nc.sync.dma_start(out=wf2, in_=wf2_d.ap())
                    g_fm = g_pool.tile([128, MC_H, 512], F8)
                else:
                    wf2 = [wm_pool.tile([128, MC_H, 384], BF16, name=f"wf2{i}")
                           for i in range(2)]
                    for nv in range(2):
                        nc.sync.dma_start(
                            out=wf2[nv],
                            in_=wf2_3[:, :, nv * 384:(nv + 1) * 384])
                    g_fm = g_pool.tile([128, MC_H, 512], BF16)

                # attention half 1, proj/LN2 of half 0 interleaved
                mv2 = {}
                pend = None
                for j in range(6):
                    if pend is not None:
                        emit_ctx(pend[0], 1, pend[1])
                    ph = emit_s_exp(j, 1)
                    pend = (j, ph)
                    if j >= 2:
                        emit_proj(j - 2)
                        mv2[j - 2] = ln_stats(j - 2)
                emit_ctx(pend[0], 1, pend[1])
                for t in range(4):
                    ln_transpose(t, h2_fm, "vec", mv2[t])
                    emit_proj(t + 4)
                for t in range(4, 8):
                    ln_transpose(t, h2_fm, "vec")

                # MLP (single table switch to gelu before the first fc1)
                for m in range(MC_H):
                    ps = emit_fc1(m, 0, w1)
                    emit_gelu(m, 0, ps, g_fm)
                for t in range(4):
                    emit_fc2(t, g_fm, wf2)
                for m in range(MC_H):
                    ps = emit_fc1(m, 1, w1)
                    emit_gelu(m, 1, ps, g_fm)
                for t in range(4, 8):
                    emit_fc2(t, g_fm, wf2)

    nc.compile()
    return nc


FC2_FP8 = False


def host_prep(x, ln1_g, ln1_b, qkv_w, proj_w, proj_b, ln2_g, ln2_b,
              fc1_w, fc1_b, fc2_w, fc2_b, fc2_fp8=FC2_FP8):
    """Fold LN affines into weights, quantize to fp8e4 (x32) / bf16."""
    import ml_dtypes
    f32 = np.float32
    bf16 = ml_dtypes.bfloat16
    f8 = ml_dtypes.float8_e4m3  # TRN e4m3: bias 7, max 240

    def q8(a):
        return np.ascontiguousarray(
            np.clip(a * WS, -240.0, 240.0).astype(f8))

    def drswi_pack(wt):
        # [768, n] -> [128, KP, n//128, 256]: il[p,kp,m,2k+i] =
        # wt[(2kp+i)*128+p, m*128+127-k] (A/B interleaved, cols reversed)
        n = wt.shape[1]
        a = wt.reshape(KP, 2, 128, n // 128, 128)[:, :, :, :, ::-1]
        return np.ascontiguousarray(
            a.transpose(2, 0, 3, 4, 1).reshape(128, KP, n // 128, 256))

    def dr_pack(wt):
        # [kin, n out] -> [128, kin//256, 2, n], in-feature = (2kp+i)*128+p
        kin, n = wt.shape
        return np.ascontiguousarray(
            wt.reshape(kin // 256, 2, 128, n).transpose(2, 0, 1, 3))

    qkv_w = np.asarray(qkv_w, f32)
    qkv_wt = (qkv_w * np.asarray(ln1_g, f32)[None, :]).T  # [768, 2304]
    qkv_bias = qkv_w @ np.asarray(ln1_b, f32)
    wqk8 = q8(drswi_pack(qkv_wt[:, :2 * DIM]))
    wv8 = q8(dr_pack(qkv_wt[:, 2 * DIM:]))
    qkb_pm = np.ascontiguousarray(qkv_bias[:2 * DIM].reshape(2 * FC, 128).T)
    vb_row = np.ascontiguousarray(
        (WS * qkv_bias[2 * DIM:]).astype(bf16).reshape(1, DIM))

    proj_wt = np.ascontiguousarray(np.asarray(proj_w, f32).T)
    wp8 = q8(dr_pack(proj_wt))
    projb_row = np.ascontiguousarray(
        (WS * WS * np.asarray(proj_b, f32)).astype(bf16).reshape(1, DIM))

    fc1_w = np.asarray(fc1_w, f32)
    fc1_wt = (fc1_w * np.asarray(ln2_g, f32)[None, :]).T  # [768, 3072]
    w18 = q8(drswi_pack(fc1_wt))
    fc1_bias = fc1_w @ np.asarray(ln2_b, f32) + np.asarray(fc1_b, f32)
    fc1b_pm = np.ascontiguousarray(fc1_bias.reshape(MC_H, 128).T)

    fc2_wt = np.ascontiguousarray(np.asarray(fc2_w, f32).T)  # [3072, 768]
    fc2b_row = np.ascontiguousarray(
        np.asarray(fc2_b, f32).astype(bf16).reshape(1, DIM))

    w = {
        "wqk8i": wqk8, "wv8": wv8, "wp8": wp8, "w18i": w18,
        "qkb_pm": qkb_pm, "fc1b_pm": fc1b_pm,
    }
    if fc2_fp8:
        w["wf28"] = q8(dr_pack(fc2_wt))
    else:
        w["wf2"] = np.ascontiguousarray(fc2_wt.astype(bf16))

    has_bias = not (
        np.all(qkv_bias[2 * DIM:] == 0.0)
        and np.all(np.asarray(proj_b, f32) == 0.0)
        and np.all(np.asarray(fc2_b, f32) == 0.0))
    if has_bias:
        w["vb_row"] = vb_row
        w["projb_row"] = projb_row
        w["fc2b_row"] = fc2b_row
    return w, has_bias


_CACHE = {}


def kernel(x, ln1_g, ln1_b, qkv_w, proj_w, proj_b, ln2_g, ln2_b,
           fc1_w, fc1_b, fc2_w, fc2_b, _want_results=False, **_ignored):
    from concourse.bass_utils import run_bass_kernel_spmd

    x = np.asarray(x, np.float32)
    B = x.shape[0]
    assert B == 8 and x.shape[1] == N_TOK and x.shape[2] == DIM

    w, has_bias = host_prep(x, ln1_g, ln1_b, qkv_w, proj_w, proj_b, ln2_g,
                            ln2_b, fc1_w, fc1_b, fc2_w, fc2_b)

    key = ("nc", has_bias, FC2_FP8)
    if key not in _CACHE:
        _CACHE[key] = build_bass(with_bias=has_bias, fc2_fp8=FC2_FP8)
        _CACHE["nc"] = _CACHE[key]
    nc = _CACHE[key]

    in_maps = [dict(w, x=np.ascontiguousarray(x[i])) for i in range(B)]
    res = run_bass_kernel_spmd(nc, in_maps, core_ids=list(range(B)))
    out = np.stack([res.results[i]["out"] for i in range(B)], axis=0)
    if _want_results:
        return out, res
    return out


# revision 21
# speedup vs baseline: 1.4629x; 1.2257x over previous
"""Trainium2 Bass kernel for a dense transformer block (pre-LN, 12 heads, MLP 4x).

Strategy: data-parallel over batch across the 8 NeuronCores (B=8 -> one batch
element per core, no collectives). Per core:

  - residual stream token-major f32 [128 tok x 8 chunks x 768] (DMA'd straight
    from HBM, no cast)
  - LN on DVE (bn_stats/bn_aggr); rstd = Exp(-0.5*Ln(var+eps)) on ACT so the
    whole kernel needs only the natural_log_exp table set plus one gelu load
  - fp8e4 (TRN e4m3, max 240) DoubleRow matmuls for qkv, ctx (E@V), proj and
    fc1: weights host-quantized at 32x scale, activations at unit scale;
    descales fold into the consuming DVE/ACT op. S (q@k) stays bf16; fc2 is
    fp8-DR or bf16 depending on the error budget (fc2_fp8 flag).
  - S (q@k, contraction = head_dim 64) uses PE row tiling: the two heads of a
    pair sit at partitions 0:64 / 64:128, so their matmuls land in row groups
    h0/h64 and run CONCURRENTLY in the PE array when interleaved - that is the
    point of the (i, ab) emission order in emit_s_exp.
  - softmax: S psum tiles [128, 2kc, 512q]; one exp per tile writing fp8 in
    the pair layout the DoubleRow ctx matmul consumes. Denominator via a
    1/32-scaled ones column in the stationary V operand; odd heads of a pair
    put v in cols 129..192 so their ctx lands on partitions 64..127 with the
    denominator on partition 0 - no cross-partition fixups. Normalization:
    reciprocal_approx_fast + gpsimd partition_broadcast + one DVE mult.
  - attention is processed in two q-halves so ctx/proj of half 0 overlap the
    exp stream of half 1. fc1+gelu+fc2 sit after the last exp because the
    gelu table set must not interleave with the exp set (2.7us reload per
    flap) and fc1 psum tiles serialize against their gelu.
  - bias matmuls (ones-row trick) are only emitted when the corresponding
    biases are nonzero; for this problem's inputs they are all zero.
"""

import numpy as np

import concourse.bass as bass
import concourse.mybir as mybir
import concourse.tile as tile
from concourse import bacc
from concourse.masks import make_identity

DIM = 768
HEADS = 12
HD = 64
HIDDEN = 3072
N_TOK = 1024
TC = N_TOK // 128  # 8 token chunks
FC = DIM // 128  # 6 feature chunks
KP = 3  # DoubleRow contraction pairs over 768
KP2 = HIDDEN // 256  # 12 DoubleRow pairs over 3072
MC_H = HIDDEN // 128  # 24 hidden chunks
EPS = 1e-5
SCALE = HD ** -0.5
WS = 32.0  # fp8 weight upscale
VP = 208  # per-head-pair free-dim stride in v_pair

F32 = mybir.dt.float32
BF16 = mybir.dt.bfloat16
F8 = mybir.dt.float8e4
DR = mybir.MatmulPerfMode.DoubleRow
DRI = mybir.MatmulPerfMode.DoubleRowSwInterleave


def build_bass(with_bias=True, fc2_fp8=False):
    nc = bacc.Bacc("TRN2", debug=False)

    x_d = nc.dram_tensor("x", [N_TOK, DIM], F32, kind="ExternalInput")
    wqk_d = nc.dram_tensor("wqk8i", [128, KP, 2 * FC, 256], F8, kind="ExternalInput")
    wv_d = nc.dram_tensor("wv8", [128, KP, 2, DIM], F8, kind="ExternalInput")
    wp_d = nc.dram_tensor("wp8", [128, KP, 2, DIM], F8, kind="ExternalInput")
    w1_d = nc.dram_tensor("w18i", [128, KP, MC_H, 256], F8, kind="ExternalInput")
    if fc2_fp8:
        wf2_d = nc.dram_tensor("wf28", [128, KP2, 2, DIM], F8, kind="ExternalInput")
    else:
        wf2_d = nc.dram_tensor("wf2", [HIDDEN, DIM], BF16, kind="ExternalInput")
    qkb_d = nc.dram_tensor("qkb_pm", [128, 2 * FC], F32, kind="ExternalInput")
    f1b_d = nc.dram_tensor("fc1b_pm", [128, MC_H], F32, kind="ExternalInput")
    if with_bias:
        vb_d = nc.dram_tensor("vb_row", [1, DIM], BF16, kind="ExternalInput")
        pb_d = nc.dram_tensor("projb_row", [1, DIM], BF16, kind="ExternalInput")
        f2b_d = nc.dram_tensor("fc2b_row", [1, DIM], BF16, kind="ExternalInput")
    out_d = nc.dram_tensor("out", [N_TOK, DIM], F32, kind="ExternalOutput")

    x_dt = x_d.ap().rearrange("(t p) c -> p t c", p=128)
    out_dt = out_d.ap().rearrange("(t p) c -> p t c", p=128)
    if not fc2_fp8:
        wf2_3 = wf2_d.ap().rearrange("(ko p) n -> p ko n", p=128)

    with tile.TileContext(nc) as tc:
        with (
            tc.tile_pool(name="const", bufs=1) as const_pool,
            tc.tile_pool(name="resid", bufs=1) as resid_pool,
            tc.tile_pool(name="stats", bufs=4) as stat_pool,
            tc.tile_pool(name="wsmall", bufs=1) as ws_pool,
            tc.tile_pool(name="h2fm", bufs=1) as h2_pool,
            tc.tile_pool(name="qk", bufs=1) as qk_pool,
            tc.tile_pool(name="vp", bufs=1) as v_pool,
            tc.tile_pool(name="ctxp", bufs=1) as ctx_pool,
            tc.tile_pool(name="exps", bufs=1) as e_pool,
            tc.tile_pool(name="dsmall", bufs=2) as d_pool,
            tc.tile_pool(name="outt", bufs=2) as o_pool,
            # PSUM: 2x[128,2,512] (4 banks) + 2x[128,512] (2) + 2x[128,512] (2)
            tc.tile_pool(name="psum_s", bufs=2, space="PSUM") as psum_s,
            tc.tile_pool(name="psum_sm", bufs=2, space="PSUM") as psum_sm,
            tc.tile_pool(name="psum_cp", bufs=2, space="PSUM") as psum_cp,
        ):
            # ---------------- constants + small weights ----------------
            ident = const_pool.tile([128, 128], BF16)
            make_identity(nc, ident)

            qkb = const_pool.tile([128, 2 * FC], F32)
            nc.sync.dma_start(out=qkb, in_=qkb_d.ap())
            f1b = const_pool.tile([128, MC_H], F32)
            nc.sync.dma_start(out=f1b, in_=f1b_d.ap())
            if with_bias:
                ones_bf = const_pool.tile([1, 128], BF16)
                nc.vector.memset(ones_bf, 1.0)
                vb_row = const_pool.tile([1, DIM], BF16)
                nc.sync.dma_start(out=vb_row, in_=vb_d.ap())
                pb_row = const_pool.tile([1, DIM], BF16)
                nc.sync.dma_start(out=pb_row, in_=pb_d.ap())
                f2b_row = const_pool.tile([1, DIM], BF16)
                nc.sync.dma_start(out=f2b_row, in_=f2b_d.ap())
            wp = ws_pool.tile([128, KP, 2, DIM], F8, name="wp")
            nc.sync.dma_start(out=wp, in_=wp_d.ap())

            x_sb = resid_pool.tile([128, TC, DIM], F32)
            h2_fm = h2_pool.tile([128, FC, N_TOK], F8)
            qk_fm = qk_pool.tile([128, 2 * FC, N_TOK], BF16)
            ctx_fm = ctx_pool.tile([128, FC, N_TOK], F8)
            # v_pair[:, t, j, :]: [0:64] v of head 2j | [64] 1/32 (den even) |
            # [65] 1/32 (den odd) | [66:129] zeros | [129:193] v head 2j+1
            v_pair = v_pool.tile([128, TC, FC, VP], F8)
            nc.vector.memset(v_pair[:, :, :, 64:129], 0.0)
            nc.vector.memset(v_pair[:, :, :, 64:66], 1.0 / WS)
            # exp staging: [p, ab, phase, kc, q]; consecutive attention
            # rounds alternate phase so ctx(j-1) reads one phase while the
            # S/exp stream of round j fills the other.
            e_buf = e_pool.tile([128, 2, 2, 8, 512], F8)

            # ---------------- helpers ----------------
            def ln_stats(t):
                st = stat_pool.tile([128, 3, 6], F32, tag="lnst")
                for sg in range(3):
                    nc.vector.bn_stats(
                        out=st[:, sg, :],
                        in_=x_sb[:, t, sg * 256:(sg + 1) * 256])
                mv = stat_pool.tile([128, 2], F32, tag="lnmv")
                nc.vector.bn_aggr(out=mv, in_=st)
                return mv

            def ln_chunk(t, h16, mv=None, h16_eng="vec"):
                # rstd = rsqrt(var+eps) entirely on DVE (magic-number seed +
                # one Newton step, ~0.1% rel err, far below the fp8 noise) so
                # the ACT engine never needs the sqrt/ln table sets.
                if mv is None:
                    mv = ln_stats(t)
                vh = stat_pool.tile([128, 1], F32, tag="lnv")
                nc.vector.tensor_scalar(
                    out=vh, in0=mv[:, 1:2], scalar1=EPS, scalar2=None,
                    op0=mybir.AluOpType.add)
                nh = stat_pool.tile([128, 1], F32, tag="lnn")
                nc.vector.tensor_scalar(
                    out=nh, in0=vh, scalar1=-0.5, scalar2=None,
                    op0=mybir.AluOpType.mult)
                # seed bits = magic - bits(vh)/2, computed in the float
                # domain (f32 rounding of the bit pattern is ~2^6 ulps --
                # irrelevant for a Newton seed; integer DVE tensor ops are
                # slow / restricted)
                fi = stat_pool.tile([128, 1], F32, tag="lnf")
                nc.vector.tensor_copy(
                    out=fi, in_=vh.bitcast(mybir.dt.uint32))
                t2 = stat_pool.tile([128, 1], F32, tag="lnt")
                nc.vector.tensor_scalar(
                    out=t2, in0=fi, scalar1=-0.5,
                    scalar2=float(0x5F3759DF),
                    op0=mybir.AluOpType.mult, op1=mybir.AluOpType.add)
                y = stat_pool.tile([128, 1], F32, tag="lnr")
                nc.vector.tensor_copy(
                    out=y.bitcast(mybir.dt.uint32), in_=t2)
                b = stat_pool.tile([128, 1], F32, tag="lnb")
                nc.vector.tensor_tensor(
                    out=b, in0=y, in1=y, op=mybir.AluOpType.mult)
                nc.vector.tensor_scalar(
                    out=b, in0=b, scalar1=nh, scalar2=1.5,
                    op0=mybir.AluOpType.mult, op1=mybir.AluOpType.add)
                rstd = stat_pool.tile([128, 1], F32, tag="lnr2")
                nc.vector.tensor_tensor(
                    out=rstd, in0=y, in1=b, op=mybir.AluOpType.mult)
                eng = nc.gpsimd if h16_eng == "gps" else nc.vector
                eng.tensor_scalar(
                    out=h16, in0=x_sb[:, t, :], scalar1=mv[:, 0:1],
                    scalar2=rstd,
                    op0=mybir.AluOpType.subtract, op1=mybir.AluOpType.mult)

            def ln_transpose(t, dst, copy_eng, mv=None, h16_eng="vec"):
                h16 = stat_pool.tile([128, DIM], BF16, tag="h16")
                ln_chunk(t, h16, mv, h16_eng)
                tr = psum_sm.tile([128, FC, 128], BF16, tag="sm")
                for f in range(FC):
                    nc.tensor.transpose(
                        tr[:, f, :], h16[:, f * 128:(f + 1) * 128], ident)
                if copy_eng == "act":
                    nc.scalar.copy(
                        out=dst[:, :, t * 128:(t + 1) * 128], in_=tr)
                else:
                    nc.vector.tensor_copy(
                        out=dst[:, :, t * 128:(t + 1) * 128], in_=tr)

            def emit_v(t, h_fm, wv):
                for nv in range(2):
                    ps = psum_sm.tile([128, 384], F32, tag="sm")
                    for kp in range(KP):
                        nc.tensor.matmul(
                            ps,
                            h_fm[:, 2 * kp:2 * kp + 2, t * 128:(t + 1) * 128],
                            wv[:, kp, :, nv * 384:(nv + 1) * 384],
                            start=(kp == 0),
                            stop=(not with_bias and kp == KP - 1),
                            perf_mode=DR)
                    if with_bias:
                        nc.tensor.matmul(
                            ps, ones_bf, vb_row[0:1, nv * 384:(nv + 1) * 384],
                            start=False, stop=True)
                    pshd = ps.rearrange("p (h d) -> p h d", d=HD)
                    jsl = slice(nv * 3, nv * 3 + 3)
                    nc.vector.tensor_scalar(
                        out=v_pair[:, t, jsl, 0:HD], in0=pshd[:, 0::2, :],
                        scalar1=1.0 / WS, scalar2=None,
                        op0=mybir.AluOpType.mult)
                    nc.vector.tensor_scalar(
                        out=v_pair[:, t, jsl, 129:193], in0=pshd[:, 1::2, :],
                        scalar1=1.0 / WS, scalar2=None,
                        op0=mybir.AluOpType.mult)

            def emit_qk(m, h_fm, wqk):
                """qk_fm[:, m, :] for feature chunk m (q: m<6, k: m>=6)."""
                for q in range(2):
                    ps = psum_sm.tile([128, 512], F32, tag="sm")
                    for kp in range(KP):
                        nc.tensor.matmul(
                            ps,
                            wqk[:, kp, m, :].rearrange(
                                "p (two c) -> p two c", two=2),
                            h_fm[:, 2 * kp:2 * kp + 2, q * 512:(q + 1) * 512],
                            start=(kp == 0), stop=(kp == KP - 1),
                            perf_mode=DRI)
                    nc.vector.tensor_scalar(
                        out=qk_fm[:, m, q * 512:(q + 1) * 512], in0=ps,
                        scalar1=1.0 / WS, scalar2=qkb[:, m:m + 1],
                        op0=mybir.AluOpType.mult, op1=mybir.AluOpType.add)

            def emit_s_exp(j, half):
                """S + exp for head pair j, q-half; returns the e_buf phase.

                The two heads' stationaries live at partitions 0:64 / 64:128
                (PE row groups h0/h64), so the two matmuls of one kc chunk run
                CONCURRENTLY in the PE array. Both land in one psum tile so
                the pair stays together through the scheduler, and one exp
                drains both."""
                ph = j % 2
                for kc in range(8):
                    sp = psum_s.tile([128, 2, 512], F32, tag="s", name="sp")
                    for ab in range(2):
                        po = 64 * ab
                        nc.tensor.matmul(
                            sp[:, ab, :],
                            qk_fm[po:po + 64, 6 + j,
                                  kc * 128:(kc + 1) * 128],
                            qk_fm[po:po + 64, j,
                                  half * 512:(half + 1) * 512],
                            start=True, stop=True)
                    nc.scalar.activation(
                        out=e_buf[:, :, ph, kc, :], in_=sp,
                        func=mybir.ActivationFunctionType.Exp, scale=SCALE)
                return ph

            def emit_ctx(j, half, ph):
                qsl = slice(half * 512, (half + 1) * 512)
                for ab in range(2):
                    vsl = slice(0, 65) if ab == 0 else slice(65, 193)
                    cp = psum_cp.tile([128, 512], F32, tag="cp")
                    cpv = cp[0:65] if ab == 0 else cp
                    for kcp in range(4):
                        nc.tensor.matmul(
                            cpv, v_pair[:, 2 * kcp:2 * kcp + 2, j, vsl],
                            e_buf[:, ab, ph, 2 * kcp:2 * kcp + 2, :],
                            start=(kcp == 0), stop=(kcp == 3), perf_mode=DR)
                    den = cp[64:65, :] if ab == 0 else cp[0:1, :]
                    dcp = d_pool.tile([1, 512], F32, tag="dcp")
                    nc.vector.tensor_copy(out=dcp, in_=den)
                    rec = d_pool.tile([1, 512], F32, tag="rec")
                    nc.vector.reciprocal_approx_fast(out=rec, in_=dcp)
                    bcd = d_pool.tile([128, 512], F32, tag="bcd")
                    nc.gpsimd.partition_broadcast(bcd, rec)
                    po = 64 * ab
                    nc.vector.tensor_tensor(
                        out=ctx_fm[po:po + 64, j, qsl],
                        in0=cp[po:po + 64, :], in1=bcd[po:po + 64, :],
                        op=mybir.AluOpType.mult)

            def emit_proj(t):
                for nv in range(2):
                    ps = psum_sm.tile([128, 384], F32, tag="sm")
                    for kp in range(KP):
                        nc.tensor.matmul(
                            ps, ctx_fm[:, 2 * kp:2 * kp + 2,
                                       t * 128:(t + 1) * 128],
                            wp[:, kp, :, nv * 384:(nv + 1) * 384],
                            start=(kp == 0),
                            stop=(not with_bias and kp == KP - 1),
                            perf_mode=DR)
                    if with_bias:
                        nc.tensor.matmul(
                            ps, ones_bf, pb_row[0:1, nv * 384:(nv + 1) * 384],
                            start=False, stop=True)
                    sl = slice(nv * 384, (nv + 1) * 384)
                    pa = o_pool.tile([128, 384], F32, tag="pa")
                    nc.vector.tensor_scalar(
                        out=pa, in0=ps, scalar1=1.0 / (WS * WS), scalar2=None,
                        op0=mybir.AluOpType.mult)
                    # residual add on DVE: gpsimd must stay on the
                    # partition_broadcast microcode lib (a lib swap is ~7.5us)
                    nc.vector.tensor_tensor(
                        out=x_sb[:, t, sl], in0=pa, in1=x_sb[:, t, sl],
                        op=mybir.AluOpType.add)

            def emit_fc1(m, half, w1):
                ps = psum_s.tile([128, 512], F32, tag="s")
                for kp in range(KP):
                    nc.tensor.matmul(
                        ps,
                        w1[:, kp, m, :].rearrange(
                            "p (two c) -> p two c", two=2),
                        h2_fm[:, 2 * kp:2 * kp + 2,
                              half * 512:(half + 1) * 512],
                        start=(kp == 0), stop=(kp == KP - 1),
                        perf_mode=DRI)
                return ps

            def emit_gelu(m, half, ps, g_fm):
                # g_fm is half-buffered [128, MC_H, 512]: half 1 reuses the
                # storage of half 0 (fc2 of half 0 is emitted in between).
                nc.scalar.activation(
                    out=g_fm[:, m, :], in_=ps,
                    func=mybir.ActivationFunctionType.Gelu,
                    bias=f1b[:, m:m + 1], scale=1.0 / WS)

            def emit_fc2(t, g_fm, wf2):
                th = (t % 4) * 128
                for nv in range(2):
                    ps = psum_sm.tile([128, 384], F32, tag="sm")
                    if fc2_fp8:
                        for kp in range(KP2):
                            nc.tensor.matmul(
                                ps,
                                g_fm[:, 2 * kp:2 * kp + 2, th:th + 128],
                                wf2[:, kp, :, nv * 384:(nv + 1) * 384],
                                start=(kp == 0),
                                stop=(not with_bias and kp == KP2 - 1),
                                perf_mode=DR)
                    else:
                        for k in range(MC_H):
                            nc.tensor.matmul(
                                ps, g_fm[:, k, th:th + 128],
                                wf2[nv][:, k, :],
                                start=(k == 0),
                                stop=(not with_bias and k == MC_H - 1))
                    if with_bias:
                        nc.tensor.matmul(
                            ps, ones_bf, f2b_row[0:1, nv * 384:(nv + 1) * 384],
                            start=False, stop=True)
                    sl = slice(nv * 384, (nv + 1) * 384)
                    o_t = o_pool.tile([128, 384], F32, tag="ot")
                    if fc2_fp8:
                        nc.vector.tensor_scalar(
                            out=o_t, in0=ps, scalar1=1.0 / WS, scalar2=None,
                            op0=mybir.AluOpType.mult)
                        nc.vector.tensor_tensor(
                            out=o_t, in0=o_t, in1=x_sb[:, t, sl],
                            op=mybir.AluOpType.add)
                    else:
                        nc.vector.tensor_add(
                            out=o_t, in0=ps, in1=x_sb[:, t, sl])
                    nc.sync.dma_start(out=out_dt[:, t, sl], in_=o_t)

            # ---------------- emission ----------------
            with (
                tc.tile_pool(name="hfm", bufs=1) as hfm_pool,
                tc.tile_pool(name="wbig", bufs=1) as wb_pool,
            ):
                wqk = wb_pool.tile([128, KP, 2 * FC, 256], F8, name="wqk")
                wv = wb_pool.tile([128, KP, 2, DIM], F8, name="wv")
                h_fm = hfm_pool.tile([128, FC, N_TOK], F8)

                for t in range(TC):
                    nc.sync.dma_start(out=x_sb[:, t, :], in_=x_dt[:, t, :])
                    ln_transpose(t, h_fm, "act")
                    if t == 0:
                        nc.sync.dma_start(out=wv, in_=wv_d.ap())
                    if t == 1:
                        nc.sync.dma_start(out=wqk, in_=wqk_d.ap())
                emit_qk(6, h_fm, wqk)
                emit_qk(0, h_fm, wqk)

                # attention half 0 (v + remaining q/k streamed into rounds)
                pend = None
                for j in range(6):
                    if pend is not None:
                        emit_ctx(pend[0], 0, pend[1])
                    ph = emit_s_exp(j, 0)
                    pend = (j, ph)
                    if j == 0:
                        for t in range(TC):
                            emit_v(t, h_fm, wv)
                    if j < 5:
                        emit_qk(7 + j, h_fm, wqk)
                        emit_qk(1 + j, h_fm, wqk)
                emit_ctx(pend[0], 0, pend[1])

            with (
                tc.tile_pool(name="gfm", bufs=1) as g_pool,
                tc.tile_pool(name="wmlp", bufs=1) as wm_pool,
            ):
                w1 = wm_pool.tile([128, KP, MC_H, 256], F8, name="w1")
                nc.sync.dma_start(out=w1, in_=w1_d.ap())
                if fc2_fp8:
                    wf2 = wm_pool.tile([128, KP2, 2, DIM], F8, name="wf28")
                    nc.sync.dma_start(out=wf2, in_=wf2_d.ap())
                    g_fm = g_pool.tile([128, MC_H, 512], F8)
                else:
                    wf2 = [wm_pool.tile([128, MC_H, 384], BF16, name=f"wf2{i}")
                           for i in range(2)]
                    for nv in range(2):
                        nc.sync.dma_start(
                            out=wf2[nv],
                            in_=wf2_3[:, :, nv * 384:(nv + 1) * 384])
                    g_fm = g_pool.tile([128, MC_H, 512], BF16)

                # attention half 1, proj/LN2 of half 0 interleaved
                mv2 = {}
                pend = None
                for j in range(6):
                    if pend is not None:
                        emit_ctx(pend[0], 1, pend[1])
                    ph = emit_s_exp(j, 1)
                    pend = (j, ph)
                    if j >= 2:
                        emit_proj(j - 2)
                        mv2[j - 2] = ln_stats(j - 2)
                emit_ctx(pend[0], 1, pend[1])
                for t in range(4):
                    ln_transpose(t, h2_fm, "vec", mv2[t])
                    emit_proj(t + 4)
                for t in range(4, 8):
                    ln_transpose(t, h2_fm, "vec")

                # MLP (single table switch to gelu before the first fc1)
                for m in range(MC_H):
                    ps = emit_fc1(m, 0, w1)
                    emit_gelu(m, 0, ps, g_fm)
                for t in range(4):
                    emit_fc2(t, g_fm, wf2)
                for m in range(MC_H):
                    ps = emit_fc1(m, 1, w1)
                    emit_gelu(m, 1, ps, g_fm)
                for t in range(4, 8):
                    emit_fc2(t, g_fm, wf2)

    nc.compile()
    return nc


FC2_FP8 = False


def host_prep(x, ln1_g, ln1_b, qkv_w, proj_w, proj_b, ln2_g, ln2_b,
              fc1_w, fc1_b, fc2_w, fc2_b, fc2_fp8=FC2_FP8):
    """Fold LN affines into weights, quantize to fp8e4 (x32) / bf16."""
    import ml_dtypes
    f32 = np.float32
    bf16 = ml_dtypes.bfloat16
    f8 = ml_dtypes.float8_e4m3  # TRN e4m3: bias 7, max 240

    def q8(a):
        return np.ascontiguousarray(
            np.clip(a * WS, -240.0, 240.0).astype(f8))

    def drswi_pack(wt):
        # [768, n] -> [128, KP, n//128, 256]: il[p,kp,m,2k+i] =
        # wt[(2kp+i)*128+p, m*128+127-k] (A/B interleaved, cols reversed)
        n = wt.shape[1]
        a = wt.reshape(KP, 2, 128, n // 128, 128)[:, :, :, :, ::-1]
        return np.ascontiguousarray(
            a.transpose(2, 0, 3, 4, 1).reshape(128, KP, n // 128, 256))

    def dr_pack(wt):
        # [kin, n out] -> [128, kin//256, 2, n], in-feature = (2kp+i)*128+p
        kin, n = wt.shape
        return np.ascontiguousarray(
            wt.reshape(kin // 256, 2, 128, n).transpose(2, 0, 1, 3))

    qkv_w = np.asarray(qkv_w, f32)
    qkv_wt = (qkv_w * np.asarray(ln1_g, f32)[None, :]).T  # [768, 2304]
    qkv_bias = qkv_w @ np.asarray(ln1_b, f32)
    wqk8 = q8(drswi_pack(qkv_wt[:, :2 * DIM]))
    wv8 = q8(dr_pack(qkv_wt[:, 2 * DIM:]))
    qkb_pm = np.ascontiguousarray(qkv_bias[:2 * DIM].reshape(2 * FC, 128).T)
    vb_row = np.ascontiguousarray(
        (WS * qkv_bias[2 * DIM:]).astype(bf16).reshape(1, DIM))

    proj_wt = np.ascontiguousarray(np.asarray(proj_w, f32).T)
    wp8 = q8(dr_pack(proj_wt))
    projb_row = np.ascontiguousarray(
        (WS * WS * np.asarray(proj_b, f32)).astype(bf16).reshape(1, DIM))

    fc1_w = np.asarray(fc1_w, f32)
    fc1_wt = (fc1_w * np.asarray(ln2_g, f32)[None, :]).T  # [768, 3072]
    w18 = q8(drswi_pack(fc1_wt))
    fc1_bias = fc1_w @ np.asarray(ln2_b, f32) + np.asarray(fc1_b, f32)
    fc1b_pm = np.ascontiguousarray(fc1_bias.reshape(MC_H, 128).T)

    fc2_wt = np.ascontiguousarray(np.asarray(fc2_w, f32).T)  # [3072, 768]
    fc2b_row = np.ascontiguousarray(
        np.asarray(fc2_b, f32).astype(bf16).reshape(1, DIM))

    w = {
        "wqk8i": wqk8, "wv8": wv8, "wp8": wp8, "w18i": w18,
        "qkb_pm": qkb_pm, "fc1b_pm": fc1b_pm,
    }
    if fc2_fp8:
        w["wf28"] = q8(dr_pack(fc2_wt))
    else:
        w["wf2"] = np.ascontiguousarray(fc2_wt.astype(bf16))

    has_bias = not (
        np.all(qkv_bias[2 * DIM:] == 0.0)
        and np.all(np.asarray(proj_b, f32) == 0.0)
        and np.all(np.asarray(fc2_b, f32) == 0.0))
    if has_bias:
        w["vb_row"] = vb_row
        w["projb_row"] = projb_row
        w["fc2b_row"] = fc2b_row
    return w, has_bias


_CACHE = {}


def kernel(x, ln1_g, ln1_b, qkv_w, proj_w, proj_b, ln2_g, ln2_b,
           fc1_w, fc1_b, fc2_w, fc2_b, _want_results=False, **_ignored):
    from concourse.bass_utils import run_bass_kernel_spmd

    x = np.asarray(x, np.float32)
    B = x.shape[0]
    assert B == 8 and x.shape[1] == N_TOK and x.shape[2] == DIM

    w, has_bias = host_prep(x, ln1_g, ln1_b, qkv_w, proj_w, proj_b, ln2_g,
                            ln2_b, fc1_w, fc1_b, fc2_w, fc2_b)

    key = ("nc", has_bias, FC2_FP8)
    if key not in _CACHE:
        _CACHE[key] = build_bass(with_bias=has_bias, fc2_fp8=FC2_FP8)
        _CACHE["nc"] = _CACHE[key]
    nc = _CACHE[key]

    in_maps = [dict(w, x=np.ascontiguousarray(x[i])) for i in range(B)]
    res = run_bass_kernel_spmd(nc, in_maps, core_ids=list(range(B)))
    out = np.stack([res.results[i]["out"] for i in range(B)], axis=0)
    if _want_results:
        return out, res
    return out


# revision 22
# speedup vs baseline: 1.6216x; 1.1085x over previous
"""Trainium2 Bass kernel for a dense transformer block (pre-LN, 12 heads, MLP 4x).

Strategy: data-parallel over batch across the 8 NeuronCores (B=8 -> one batch
element per core, no collectives). Per core:

  - residual stream token-major f32 [128 tok x 8 chunks x 768] (DMA'd straight
    from HBM, no cast)
  - LN on DVE (bn_stats/bn_aggr); rstd = Exp(-0.5*Ln(var+eps)) on ACT so the
    whole kernel needs only the natural_log_exp table set plus one gelu load
  - fp8e4 (TRN e4m3, max 240) DoubleRow matmuls for qkv, ctx (E@V), proj and
    fc1: weights host-quantized at 32x scale, activations at unit scale;
    descales fold into the consuming DVE/ACT op. S (q@k) stays bf16; fc2 is
    fp8-DR or bf16 depending on the error budget (fc2_fp8 flag).
  - S (q@k, contraction = head_dim 64) uses PE row tiling: the two heads of a
    pair sit at partitions 0:64 / 64:128, so their matmuls land in row groups
    h0/h64 and run CONCURRENTLY in the PE array when interleaved - that is the
    point of the (i, ab) emission order in emit_s_exp.
  - softmax: S psum tiles [128, 2kc, 512q]; one exp per tile writing fp8 in
    the pair layout the DoubleRow ctx matmul consumes. Denominator via a
    1/32-scaled ones column in the stationary V operand; odd heads of a pair
    put v in cols 129..192 so their ctx lands on partitions 64..127 with the
    denominator on partition 0 - no cross-partition fixups. Normalization:
    reciprocal_approx_fast + gpsimd partition_broadcast + one DVE mult.
  - attention is processed in two q-halves so ctx/proj of half 0 overlap the
    exp stream of half 1. fc1+gelu+fc2 sit after the last exp because the
    gelu table set must not interleave with the exp set (2.7us reload per
    flap) and fc1 psum tiles serialize against their gelu.
  - bias matmuls (ones-row trick) are only emitted when the corresponding
    biases are nonzero; for this problem's inputs they are all zero.
"""

import numpy as np

import concourse.bass as bass
import concourse.mybir as mybir
import concourse.tile as tile
from concourse import bacc
from concourse.masks import make_identity

DIM = 768
HEADS = 12
HD = 64
HIDDEN = 3072
N_TOK = 1024
TC = N_TOK // 128  # 8 token chunks
FC = DIM // 128  # 6 feature chunks
KP = 3  # DoubleRow contraction pairs over 768
KP2 = HIDDEN // 256  # 12 DoubleRow pairs over 3072
MC_H = HIDDEN // 128  # 24 hidden chunks
EPS = 1e-5
SCALE = HD ** -0.5
WS = 32.0  # fp8 weight upscale
VP = 208  # per-head-pair free-dim stride in v_pair

F32 = mybir.dt.float32
BF16 = mybir.dt.bfloat16
F8 = mybir.dt.float8e4
DR = mybir.MatmulPerfMode.DoubleRow
DRI = mybir.MatmulPerfMode.DoubleRowSwInterleave


def build_bass(with_bias=True, fc2_fp8=False):
    nc = bacc.Bacc("TRN2", debug=False)

    x_d = nc.dram_tensor("x", [N_TOK, DIM], F32, kind="ExternalInput")
    wqk_d = nc.dram_tensor("wqk8i", [128, KP, 2 * FC, 256], F8, kind="ExternalInput")
    wv_d = nc.dram_tensor("wv8", [128, KP, 2, DIM], F8, kind="ExternalInput")
    wp_d = nc.dram_tensor("wp8", [128, KP, 2, DIM], F8, kind="ExternalInput")
    w1_d = nc.dram_tensor("w18i", [128, KP, MC_H, 256], F8, kind="ExternalInput")
    if fc2_fp8:
        wf2_d = nc.dram_tensor("wf28", [128, KP2, 2, DIM], F8, kind="ExternalInput")
    else:
        wf2_d = nc.dram_tensor("wf2", [HIDDEN, DIM], BF16, kind="ExternalInput")
    qkb_d = nc.dram_tensor("qkb_pm", [128, 2 * FC], F32, kind="ExternalInput")
    f1b_d = nc.dram_tensor("fc1b_pm", [128, MC_H], F32, kind="ExternalInput")
    if with_bias:
        vb_d = nc.dram_tensor("vb_row", [1, DIM], BF16, kind="ExternalInput")
        pb_d = nc.dram_tensor("projb_row", [1, DIM], BF16, kind="ExternalInput")
        f2b_d = nc.dram_tensor("fc2b_row", [1, DIM], BF16, kind="ExternalInput")
    out_d = nc.dram_tensor("out", [N_TOK, DIM], F32, kind="ExternalOutput")

    x_dt = x_d.ap().rearrange("(t p) c -> p t c", p=128)
    out_dt = out_d.ap().rearrange("(t p) c -> p t c", p=128)
    if not fc2_fp8:
        wf2_3 = wf2_d.ap().rearrange("(ko p) n -> p ko n", p=128)

    with tile.TileContext(nc) as tc:
        with (
            tc.tile_pool(name="const", bufs=1) as const_pool,
            tc.tile_pool(name="resid", bufs=1) as resid_pool,
            tc.tile_pool(name="stats", bufs=4) as stat_pool,
            tc.tile_pool(name="wsmall", bufs=1) as ws_pool,
            tc.tile_pool(name="h2fm", bufs=1) as h2_pool,
            tc.tile_pool(name="qk", bufs=1) as qk_pool,
            tc.tile_pool(name="vp", bufs=1) as v_pool,
            tc.tile_pool(name="ctxp", bufs=1) as ctx_pool,
            tc.tile_pool(name="exps", bufs=1) as e_pool,
            tc.tile_pool(name="dsmall", bufs=2) as d_pool,
            tc.tile_pool(name="outt", bufs=2) as o_pool,
            # PSUM: 2x[128,2,512] (4 banks) + 2x[128,512] (2) + 2x[128,512] (2)
            tc.tile_pool(name="psum_s", bufs=2, space="PSUM") as psum_s,
            tc.tile_pool(name="psum_sm", bufs=2, space="PSUM") as psum_sm,
            tc.tile_pool(name="psum_cp", bufs=2, space="PSUM") as psum_cp,
        ):
            # ---------------- constants + small weights ----------------
            ident = const_pool.tile([128, 128], BF16)
            make_identity(nc, ident)

            qkb = const_pool.tile([128, 2 * FC], F32)
            nc.sync.dma_start(out=qkb, in_=qkb_d.ap())
            f1b = const_pool.tile([128, MC_H], F32)
            nc.sync.dma_start(out=f1b, in_=f1b_d.ap())
            if with_bias:
                ones_bf = const_pool.tile([1, 128], BF16)
                nc.vector.memset(ones_bf, 1.0)
                vb_row = const_pool.tile([1, DIM], BF16)
                nc.sync.dma_start(out=vb_row, in_=vb_d.ap())
                pb_row = const_pool.tile([1, DIM], BF16)
                nc.sync.dma_start(out=pb_row, in_=pb_d.ap())
                f2b_row = const_pool.tile([1, DIM], BF16)
                nc.sync.dma_start(out=f2b_row, in_=f2b_d.ap())
            wp = ws_pool.tile([128, KP, 2, DIM], F8, name="wp")
            nc.sync.dma_start(out=wp, in_=wp_d.ap())

            x_sb = resid_pool.tile([128, TC, DIM], F32)
            h2_fm = h2_pool.tile([128, FC, N_TOK], F8)
            qk_fm = qk_pool.tile([128, 2 * FC, N_TOK], BF16)
            ctx_fm = ctx_pool.tile([128, FC, N_TOK], F8)
            # v_pair[:, t, j, :]: [0:64] v of head 2j | [64] 1/32 (den even) |
            # [65] 1/32 (den odd) | [66:129] zeros | [129:193] v head 2j+1
            v_pair = v_pool.tile([128, TC, FC, VP], F8)
            nc.vector.memset(v_pair[:, :, :, 64:129], 0.0)
            nc.vector.memset(v_pair[:, :, :, 64:66], 1.0 / WS)
            # exp staging: [p, ab, phase, kc, q]; consecutive attention
            # rounds alternate phase so ctx(j-1) reads one phase while the
            # S/exp stream of round j fills the other.
            e_buf = e_pool.tile([128, 2, 2, 8, 512], F8)

            # ---------------- helpers ----------------
            def ln_stats(t):
                st = stat_pool.tile([128, 3, 6], F32, tag="lnst")
                for sg in range(3):
                    nc.vector.bn_stats(
                        out=st[:, sg, :],
                        in_=x_sb[:, t, sg * 256:(sg + 1) * 256])
                mv = stat_pool.tile([128, 2], F32, tag="lnmv")
                nc.vector.bn_aggr(out=mv, in_=st)
                return mv

            def ln_chunk(t, h16, mv=None, h16_eng="vec"):
                # rstd = rsqrt(var+eps) entirely on DVE (magic-number seed +
                # one Newton step, ~0.1% rel err, far below the fp8 noise) so
                # the ACT engine never needs the sqrt/ln table sets.
                if mv is None:
                    mv = ln_stats(t)
                vh = stat_pool.tile([128, 1], F32, tag="lnv")
                nc.vector.tensor_scalar(
                    out=vh, in0=mv[:, 1:2], scalar1=EPS, scalar2=None,
                    op0=mybir.AluOpType.add)
                nh = stat_pool.tile([128, 1], F32, tag="lnn")
                nc.vector.tensor_scalar(
                    out=nh, in0=vh, scalar1=-0.5, scalar2=None,
                    op0=mybir.AluOpType.mult)
                # seed bits = magic - bits(vh)/2, computed in the float
                # domain (f32 rounding of the bit pattern is ~2^6 ulps --
                # irrelevant for a Newton seed; integer DVE tensor ops are
                # slow / restricted)
                fi = stat_pool.tile([128, 1], F32, tag="lnf")
                nc.vector.tensor_copy(
                    out=fi, in_=vh.bitcast(mybir.dt.uint32))
                t2 = stat_pool.tile([128, 1], F32, tag="lnt")
                nc.vector.tensor_scalar(
                    out=t2, in0=fi, scalar1=-0.5,
                    scalar2=float(0x5F3759DF),
                    op0=mybir.AluOpType.mult, op1=mybir.AluOpType.add)
                y = stat_pool.tile([128, 1], F32, tag="lnr")
                nc.vector.tensor_copy(
                    out=y.bitcast(mybir.dt.uint32), in_=t2)
                b = stat_pool.tile([128, 1], F32, tag="lnb")
                nc.vector.tensor_tensor(
                    out=b, in0=y, in1=y, op=mybir.AluOpType.mult)
                nc.vector.tensor_scalar(
                    out=b, in0=b, scalar1=nh, scalar2=1.5,
                    op0=mybir.AluOpType.mult, op1=mybir.AluOpType.add)
                rstd = stat_pool.tile([128, 1], F32, tag="lnr2")
                nc.vector.tensor_tensor(
                    out=rstd, in0=y, in1=b, op=mybir.AluOpType.mult)
                eng = nc.gpsimd if h16_eng == "gps" else nc.vector
                eng.tensor_scalar(
                    out=h16, in0=x_sb[:, t, :], scalar1=mv[:, 0:1],
                    scalar2=rstd,
                    op0=mybir.AluOpType.subtract, op1=mybir.AluOpType.mult)

            def ln_transpose(t, dst, copy_eng, mv=None, h16_eng="vec"):
                h16 = stat_pool.tile([128, DIM], BF16, tag="h16")
                ln_chunk(t, h16, mv, h16_eng)
                tr = psum_sm.tile([128, FC, 128], BF16, tag="sm")
                for f in range(FC):
                    nc.tensor.transpose(
                        tr[:, f, :], h16[:, f * 128:(f + 1) * 128], ident)
                if copy_eng == "act":
                    nc.scalar.copy(
                        out=dst[:, :, t * 128:(t + 1) * 128], in_=tr)
                else:
                    nc.vector.tensor_copy(
                        out=dst[:, :, t * 128:(t + 1) * 128], in_=tr)

            def emit_v(t, h_fm, wv):
                for nv in range(2):
                    ps = psum_sm.tile([128, 384], F32, tag="sm")
                    for kp in range(KP):
                        nc.tensor.matmul(
                            ps,
                            h_fm[:, 2 * kp:2 * kp + 2, t * 128:(t + 1) * 128],
                            wv[:, kp, :, nv * 384:(nv + 1) * 384],
                            start=(kp == 0),
                            stop=(not with_bias and kp == KP - 1),
                            perf_mode=DR)
                    if with_bias:
                        nc.tensor.matmul(
                            ps, ones_bf, vb_row[0:1, nv * 384:(nv + 1) * 384],
                            start=False, stop=True)
                    pshd = ps.rearrange("p (h d) -> p h d", d=HD)
                    jsl = slice(nv * 3, nv * 3 + 3)
                    nc.vector.tensor_scalar(
                        out=v_pair[:, t, jsl, 0:HD], in0=pshd[:, 0::2, :],
                        scalar1=1.0 / WS, scalar2=None,
                        op0=mybir.AluOpType.mult)
                    nc.vector.tensor_scalar(
                        out=v_pair[:, t, jsl, 129:193], in0=pshd[:, 1::2, :],
                        scalar1=1.0 / WS, scalar2=None,
                        op0=mybir.AluOpType.mult)

            def emit_qk(m, h_fm, wqk):
                """qk_fm[:, m, :] for feature chunk m (q: m<6, k: m>=6)."""
                for q in range(2):
                    ps = psum_sm.tile([128, 512], F32, tag="sm")
                    for kp in range(KP):
                        nc.tensor.matmul(
                            ps,
                            wqk[:, kp, m, :].rearrange(
                                "p (two c) -> p two c", two=2),
                            h_fm[:, 2 * kp:2 * kp + 2, q * 512:(q + 1) * 512],
                            start=(kp == 0), stop=(kp == KP - 1),
                            perf_mode=DRI)
                    nc.vector.tensor_scalar(
                        out=qk_fm[:, m, q * 512:(q + 1) * 512], in0=ps,
                        scalar1=1.0 / WS, scalar2=qkb[:, m:m + 1],
                        op0=mybir.AluOpType.mult, op1=mybir.AluOpType.add)

            def emit_s_exp(j, half):
                """S + exp for head pair j, q-half; returns the e_buf phase.

                The two heads' stationaries live at partitions 0:64 / 64:128
                (PE row groups h0/h64), so the two matmuls of one kc chunk run
                CONCURRENTLY in the PE array. Both land in one psum tile so
                the pair stays together through the scheduler, and one exp
                drains both."""
                ph = j % 2
                for kc in range(8):
                    sp = psum_s.tile([128, 2, 512], F32, tag="s", name="sp")
                    for ab in range(2):
                        po = 64 * ab
                        nc.tensor.matmul(
                            sp[:, ab, :],
                            qk_fm[po:po + 64, 6 + j,
                                  kc * 128:(kc + 1) * 128],
                            qk_fm[po:po + 64, j,
                                  half * 512:(half + 1) * 512],
                            start=True, stop=True)
                    nc.scalar.activation(
                        out=e_buf[:, :, ph, kc, :], in_=sp,
                        func=mybir.ActivationFunctionType.Exp, scale=SCALE)
                return ph

            def emit_ctx(j, half, ph):
                qsl = slice(half * 512, (half + 1) * 512)
                for ab in range(2):
                    vsl = slice(0, 65) if ab == 0 else slice(65, 193)
                    cp = psum_cp.tile([128, 512], F32, tag="cp")
                    cpv = cp[0:65] if ab == 0 else cp
                    for kcp in range(4):
                        nc.tensor.matmul(
                            cpv, v_pair[:, 2 * kcp:2 * kcp + 2, j, vsl],
                            e_buf[:, ab, ph, 2 * kcp:2 * kcp + 2, :],
                            start=(kcp == 0), stop=(kcp == 3), perf_mode=DR)
                    den = cp[64:65, :] if ab == 0 else cp[0:1, :]
                    dcp = d_pool.tile([1, 512], F32, tag="dcp")
                    nc.vector.tensor_copy(out=dcp, in_=den)
                    rec = d_pool.tile([1, 512], F32, tag="rec")
                    nc.vector.reciprocal_approx_fast(out=rec, in_=dcp)
                    bcd = d_pool.tile([128, 512], F32, tag="bcd")
                    nc.gpsimd.partition_broadcast(bcd, rec)
                    po = 64 * ab
                    nc.vector.tensor_tensor(
                        out=ctx_fm[po:po + 64, j, qsl],
                        in0=cp[po:po + 64, :], in1=bcd[po:po + 64, :],
                        op=mybir.AluOpType.mult)

            def emit_proj(t):
                for nv in range(2):
                    ps = psum_sm.tile([128, 384], F32, tag="sm")
                    for kp in range(KP):
                        nc.tensor.matmul(
                            ps, ctx_fm[:, 2 * kp:2 * kp + 2,
                                       t * 128:(t + 1) * 128],
                            wp[:, kp, :, nv * 384:(nv + 1) * 384],
                            start=(kp == 0),
                            stop=(not with_bias and kp == KP - 1),
                            perf_mode=DR)
                    if with_bias:
                        nc.tensor.matmul(
                            ps, ones_bf, pb_row[0:1, nv * 384:(nv + 1) * 384],
                            start=False, stop=True)
                    sl = slice(nv * 384, (nv + 1) * 384)
                    pa = o_pool.tile([128, 384], F32, tag="pa")
                    nc.vector.tensor_scalar(
                        out=pa, in0=ps, scalar1=1.0 / (WS * WS), scalar2=None,
                        op0=mybir.AluOpType.mult)
                    # residual add on DVE: gpsimd must stay on the
                    # partition_broadcast microcode lib (a lib swap is ~7.5us)
                    nc.vector.tensor_tensor(
                        out=x_sb[:, t, sl], in0=pa, in1=x_sb[:, t, sl],
                        op=mybir.AluOpType.add)

            def emit_fc1(m, half, w1):
                ps = psum_s.tile([128, 512], F32, tag="s")
                for kp in range(KP):
                    nc.tensor.matmul(
                        ps,
                        w1[:, kp, m, :].rearrange(
                            "p (two c) -> p two c", two=2),
                        h2_fm[:, 2 * kp:2 * kp + 2,
                              half * 512:(half + 1) * 512],
                        start=(kp == 0), stop=(kp == KP - 1),
                        perf_mode=DRI)
                return ps

            def emit_gelu(m, half, ps, g_fm):
                # g_fm is half-buffered [128, MC_H, 512]: half 1 reuses the
                # storage of half 0 (fc2 of half 0 is emitted in between).
                nc.scalar.activation(
                    out=g_fm[:, m, :], in_=ps,
                    func=mybir.ActivationFunctionType.Gelu,
                    bias=f1b[:, m:m + 1], scale=1.0 / WS)

            def emit_fc2(t, g_fm, wf2):
                th = (t % 4) * 128
                for nv in range(2):
                    ps = psum_sm.tile([128, 384], F32, tag="sm")
                    if fc2_fp8:
                        for kp in range(KP2):
                            nc.tensor.matmul(
                                ps,
                                g_fm[:, 2 * kp:2 * kp + 2, th:th + 128],
                                wf2[:, kp, :, nv * 384:(nv + 1) * 384],
                                start=(kp == 0),
                                stop=(not with_bias and kp == KP2 - 1),
                                perf_mode=DR)
                    else:
                        for k in range(MC_H):
                            nc.tensor.matmul(
                                ps, g_fm[:, k, th:th + 128],
                                wf2[nv][:, k, :],
                                start=(k == 0),
                                stop=(not with_bias and k == MC_H - 1))
                    if with_bias:
                        nc.tensor.matmul(
                            ps, ones_bf, f2b_row[0:1, nv * 384:(nv + 1) * 384],
                            start=False, stop=True)
                    sl = slice(nv * 384, (nv + 1) * 384)
                    o_t = o_pool.tile([128, 384], F32, tag="ot")
                    if fc2_fp8:
                        nc.vector.tensor_scalar(
                            out=o_t, in0=ps, scalar1=1.0 / WS, scalar2=None,
                            op0=mybir.AluOpType.mult)
                        nc.vector.tensor_tensor(
                            out=o_t, in0=o_t, in1=x_sb[:, t, sl],
                            op=mybir.AluOpType.add)
                    else:
                        nc.vector.tensor_add(
                            out=o_t, in0=ps, in1=x_sb[:, t, sl])
                    nc.sync.dma_start(out=out_dt[:, t, sl], in_=o_t)

            # ---------------- emission ----------------
            with (
                tc.tile_pool(name="hfm", bufs=1) as hfm_pool,
                tc.tile_pool(name="wbig", bufs=1) as wb_pool,
            ):
                wqk = wb_pool.tile([128, KP, 2 * FC, 256], F8, name="wqk")
                wv = wb_pool.tile([128, KP, 2, DIM], F8, name="wv")
                h_fm = hfm_pool.tile([128, FC, N_TOK], F8)

                for t in range(TC):
                    nc.sync.dma_start(out=x_sb[:, t, :], in_=x_dt[:, t, :])
                    ln_transpose(t, h_fm, "act")
                    if t == 0:
                        nc.sync.dma_start(out=wv, in_=wv_d.ap())
                    if t == 1:
                        nc.sync.dma_start(out=wqk, in_=wqk_d.ap())
                emit_qk(6, h_fm, wqk)
                emit_qk(0, h_fm, wqk)

                # attention half 0 (v + remaining q/k streamed into rounds)
                pend = None
                for j in range(6):
                    if pend is not None:
                        emit_ctx(pend[0], 0, pend[1])
                    ph = emit_s_exp(j, 0)
                    pend = (j, ph)
                    if j == 0:
                        for t in range(TC):
                            emit_v(t, h_fm, wv)
                    if j < 5:
                        emit_qk(7 + j, h_fm, wqk)
                        emit_qk(1 + j, h_fm, wqk)
                emit_ctx(pend[0], 0, pend[1])

            with (
                tc.tile_pool(name="gfm", bufs=1) as g_pool,
                tc.tile_pool(name="wmlp", bufs=1) as wm_pool,
            ):
                w1 = wm_pool.tile([128, KP, MC_H, 256], F8, name="w1")
                nc.sync.dma_start(out=w1, in_=w1_d.ap())
                if fc2_fp8:
                    wf2 = wm_pool.tile([128, KP2, 2, DIM], F8, name="wf28")
                    nc.sync.dma_start(out=wf2, in_=wf2_d.ap())
                    g_fm = g_pool.tile([128, MC_H, 512], F8)
                else:
                    wf2 = [wm_pool.tile([128, MC_H, 384], BF16, name=f"wf2{i}")
                           for i in range(2)]
                    for nv in range(2):
                        nc.sync.dma_start(
                            out=wf2[nv],
                            in_=wf2_3[:, :, nv * 384:(nv + 1) * 384])
                    g_fm = g_pool.tile([128, MC_H, 512], BF16)

                # attention half 1, proj/LN2 of half 0 interleaved
                mv2 = {}
                pend = None
                for j in range(6):
                    if pend is not None:
                        emit_ctx(pend[0], 1, pend[1])
                    ph = emit_s_exp(j, 1)
                    pend = (j, ph)
                    if j >= 2:
                        emit_proj(j - 2)
                        mv2[j - 2] = ln_stats(j - 2)
                emit_ctx(pend[0], 1, pend[1])
                for t in range(4):
                    ln_transpose(t, h2_fm, "vec", mv2[t])
                    emit_proj(t + 4)
                for t in range(4, 8):
                    ln_transpose(t, h2_fm, "vec")

                # MLP (single table switch to gelu before the first fc1)
                for m in range(MC_H):
                    ps = emit_fc1(m, 0, w1)
                    emit_gelu(m, 0, ps, g_fm)
                for t in range(4):
                    emit_fc2(t, g_fm, wf2)
                for m in range(MC_H):
                    ps = emit_fc1(m, 1, w1)
                    emit_gelu(m, 1, ps, g_fm)
                for t in range(4, 8):
                    emit_fc2(t, g_fm, wf2)

    nc.compile()
    return nc


FC2_FP8 = True


def host_prep(x, ln1_g, ln1_b, qkv_w, proj_w, proj_b, ln2_g, ln2_b,
              fc1_w, fc1_b, fc2_w, fc2_b, fc2_fp8=FC2_FP8):
    """Fold LN affines into weights, quantize to fp8e4 (x32) / bf16."""
    import ml_dtypes
    f32 = np.float32
    bf16 = ml_dtypes.bfloat16
    f8 = ml_dtypes.float8_e4m3  # TRN e4m3: bias 7, max 240

    def q8(a):
        return np.ascontiguousarray(
            np.clip(a * WS, -240.0, 240.0).astype(f8))

    def drswi_pack(wt):
        # [768, n] -> [128, KP, n//128, 256]: il[p,kp,m,2k+i] =
        # wt[(2kp+i)*128+p, m*128+127-k] (A/B interleaved, cols reversed)
        n = wt.shape[1]
        a = wt.reshape(KP, 2, 128, n // 128, 128)[:, :, :, :, ::-1]
        return np.ascontiguousarray(
            a.transpose(2, 0, 3, 4, 1).reshape(128, KP, n // 128, 256))

    def dr_pack(wt):
        # [kin, n out] -> [128, kin//256, 2, n], in-feature = (2kp+i)*128+p
        kin, n = wt.shape
        return np.ascontiguousarray(
            wt.reshape(kin // 256, 2, 128, n).transpose(2, 0, 1, 3))

    qkv_w = np.asarray(qkv_w, f32)
    qkv_wt = (qkv_w * np.asarray(ln1_g, f32)[None, :]).T  # [768, 2304]
    qkv_bias = qkv_w @ np.asarray(ln1_b, f32)
    wqk8 = q8(drswi_pack(qkv_wt[:, :2 * DIM]))
    wv8 = q8(dr_pack(qkv_wt[:, 2 * DIM:]))
    qkb_pm = np.ascontiguousarray(qkv_bias[:2 * DIM].reshape(2 * FC, 128).T)
    vb_row = np.ascontiguousarray(
        (WS * qkv_bias[2 * DIM:]).astype(bf16).reshape(1, DIM))

    proj_wt = np.ascontiguousarray(np.asarray(proj_w, f32).T)
    wp8 = q8(dr_pack(proj_wt))
    projb_row = np.ascontiguousarray(
        (WS * WS * np.asarray(proj_b, f32)).astype(bf16).reshape(1, DIM))

    fc1_w = np.asarray(fc1_w, f32)
    fc1_wt = (fc1_w * np.asarray(ln2_g, f32)[None, :]).T  # [768, 3072]
    w18 = q8(drswi_pack(fc1_wt))
    fc1_bias = fc1_w @ np.asarray(ln2_b, f32) + np.asarray(fc1_b, f32)
    fc1b_pm = np.ascontiguousarray(fc1_bias.reshape(MC_H, 128).T)

    fc2_wt = np.ascontiguousarray(np.asarray(fc2_w, f32).T)  # [3072, 768]
    fc2b_row = np.ascontiguousarray(
        np.asarray(fc2_b, f32).astype(bf16).reshape(1, DIM))

    w = {
        "wqk8i": wqk8, "wv8": wv8, "wp8": wp8, "w18i": w18,
        "qkb_pm": qkb_pm, "fc1b_pm": fc1b_pm,
    }
    if fc2_fp8:
        w["wf28"] = q8(dr_pack(fc2_wt))
    else:
        w["wf2"] = np.ascontiguousarray(fc2_wt.astype(bf16))

    has_bias = not (
        np.all(qkv_bias[2 * DIM:] == 0.0)
        and np.all(np.asarray(proj_b, f32) == 0.0)
        and np.all(np.asarray(fc2_b, f32) == 0.0))
    if has_bias:
        w["vb_row"] = vb_row
        w["projb_row"] = projb_row
        w["fc2b_row"] = fc2b_row
    return w, has_bias


_CACHE = {}


def kernel(x, ln1_g, ln1_b, qkv_w, proj_w, proj_b, ln2_g, ln2_b,
           fc1_w, fc1_b, fc2_w, fc2_b, _want_results=False, **_ignored):
    from concourse.bass_utils import run_bass_kernel_spmd

    x = np.asarray(x, np.float32)
    B = x.shape[0]
    assert B == 8 and x.shape[1] == N_TOK and x.shape[2] == DIM

    w, has_bias = host_prep(x, ln1_g, ln1_b, qkv_w, proj_w, proj_b, ln2_g,
                            ln2_b, fc1_w, fc1_b, fc2_w, fc2_b)

    key = ("nc", has_bias, FC2_FP8)
    if key not in _CACHE:
        _CACHE[key] = build_bass(with_bias=has_bias, fc2_fp8=FC2_FP8)
        _CACHE["nc"] = _CACHE[key]
    nc = _CACHE[key]

    in_maps = [dict(w, x=np.ascontiguousarray(x[i])) for i in range(B)]
    res = run_bass_kernel_spmd(nc, in_maps, core_ids=list(range(B)))
    out = np.stack([res.results[i]["out"] for i in range(B)], axis=0)
    if _want_results:
        return out, res
    return out
